# revision 1
# baseline (speedup 1.0000x reference)
"""Causal self-attention (B=1, T=2048, D=1024, H=8, hd=128) on 8 trn2 cores.

Sharding: tensor-parallel over heads -- one head per core. Each core computes
its head's qkv projection, rms-norm+rotary, causal attention, and the c_proj
partial product for its head; the host sums the 8 partial [D, T] outputs.

Numerics (measured end-to-end max-rel error ~1.2e-3 in simulation):
  - qkv projection in fp8(e4m3) DoubleRow perf mode: the host splits x and
    64*Wqkv into (hi, lo) fp8 planes; the kernel computes
    x_hi@W_hi + x_lo@W_hi + x_hi@W_lo, pairing adjacent 128-contraction
    chunks per DoubleRow matmul (256-contraction per instruction at 0.5
    cycles/row -- 2.67x the bf16 rate for the 3 terms). Wv planes carry
    lambda0, ve is prescaled by lambda1 on the host, and the PSUM drain
    scale 1/64 undoes the fp8 range prescale.
  - everything else runs in fp16 (vs bf16): qkv natural tiles, rotary,
    rms-scaled transposes, scores, probabilities, v-hat, y-hat, c_proj.
  - softmax exp is shifted by -4.5 so e^(smax) fits fp16 range; the shift
    cancels exactly in the p/sum(p) normalization.
  - rms scale r is computed with a single fused Rsqrt activation; the
    ATTN_SCALE is folded into rq, eps into the Rsqrt bias.
"""

import numpy as np

B, T, D = 1, 2048, 1024
H, HD = 8, 128
SCALE = 0.12
NCORES = 8
NT = T // 128      # 16 token tiles
NCH = D // 128     # 8 contraction chunks
NPAIR = NCH // 2   # 4 DoubleRow chunk pairs
NTJ = 4            # attention t-blocks
TJ = T // NTJ      # 512
EPS = float(np.finfo(np.float32).eps)
WS = 64.0          # fp8 weight prescale
ESH = 4.5          # exp shift

_CACHE = {}


def _bcast(ap, n):
    """Broadcast a [..., 1] AP to [..., n] via a step-0 trailing dim."""
    try:
        return ap.to_broadcast(list(ap.shape[:-1]) + [n])
    except Exception:
        import concourse.bass as bass
        return bass.AP(tensor=ap.tensor, offset=ap.offset,
                       ap=list(ap.ap[:-1]) + [[0, n]])


def _bcast_mid(ap, n):
    """Insert a step-0 middle dim: [p, f] -> [p, n, f]."""
    import concourse.bass as bass
    return bass.AP(tensor=ap.tensor, offset=ap.offset,
                   ap=[list(ap.ap[0]), [0, n], list(ap.ap[1])])


def _build_program():
    if "nc" in _CACHE:
        return _CACHE["nc"]

    import concourse.bacc as bacc
    import concourse.tile as tile
    import concourse.mybir as mybir

    f32 = mybir.dt.float32
    f16 = mybir.dt.float16
    fp8 = mybir.dt.float8e4
    AF = mybir.ActivationFunctionType
    ALU = mybir.AluOpType
    DR = mybir.MatmulPerfMode.DoubleRow

    nc = bacc.Bacc("TRN2", target_bir_lowering=False, debug=False)

    # x planes: [batch of 4 tiles, c-in-part 128, tile, chunk, token]
    xh_d = nc.dram_tensor("xh", [4, 128, 4, NCH, 128], fp8,
                          kind="ExternalInput")
    xl_d = nc.dram_tensor("xl", [4, 128, 4, NCH, 128], fp8,
                          kind="ExternalInput")
    wh_d = nc.dram_tensor("wh", [128, NCH, 3 * HD], fp8, kind="ExternalInput")
    wl_d = nc.dram_tensor("wl", [128, NCH, 3 * HD], fp8, kind="ExternalInput")
    ve_d = nc.dram_tensor("veN", [128, NT, HD], f16, kind="ExternalInput")
    cw_d = nc.dram_tensor("cwT", [HD, D], f16, kind="ExternalInput")
    cos_d = nc.dram_tensor("cosT", [128, NT, 32], f16, kind="ExternalInput")
    sin_d = nc.dram_tensor("sinT", [128, NT, 32], f16, kind="ExternalInput")
    tri_d = nc.dram_tensor("tri", [128, 128], f16, kind="ExternalInput")
    idn_d = nc.dram_tensor("idn", [128, 128], f16, kind="ExternalInput")
    out_d = nc.dram_tensor("outT", [D, T], f16, kind="ExternalOutput")

    with tile.TileContext(nc) as tc:
        with tc.tile_pool(name="const", bufs=1) as cpool, \
             tc.tile_pool(name="work", bufs=1) as wpool, \
             tc.tile_pool(name="xs", bufs=1) as xpool:
            # ---- resident inputs ----
            wh_sb = cpool.tile([128, NCH, 3 * HD], fp8)
            wl_sb = cpool.tile([128, NCH, 3 * HD], fp8)
            ve_sb = cpool.tile([128, NT, HD], f16)
            cw_sb = cpool.tile([HD, D], f16)
            cos_sb = cpool.tile([128, NT, 32], f16)
            sin_sb = cpool.tile([128, NT, 32], f16)
            tri_sb = cpool.tile([128, 128], f16)
            idn_sb = cpool.tile([128, 128], f16)
            ones_sb = cpool.tile([128, 128], f16)
            zero_sb = cpool.tile([128, 1], f32)
            bexp_sb = cpool.tile([128, 1], f32)   # exp shift bias
            bq_sb = cpool.tile([128, 1], f32)     # eps/SCALE^2 bias for rq
            bk_sb = cpool.tile([128, 1], f32)     # eps bias for rk
            sc64_sb = cpool.tile([128, 1], f32)   # 1/WS drain scale

            nc.sync.dma_start(wh_sb[:], wh_d[:])
            nc.sync.dma_start(wl_sb[:], wl_d[:])
            # x batch loads issued inside the qkv loop (token-tile major);
            # const loads are interleaved there so they don't delay x
            nc.vector.memset(ones_sb[:], 1.0)
            nc.vector.memset(zero_sb[:], 0.0)
            nc.vector.memset(bexp_sb[:], -ESH)
            nc.vector.memset(bq_sb[:], EPS / (SCALE * SCALE))
            nc.vector.memset(bk_sb[:], EPS)
            nc.vector.memset(sc64_sb[:], 1.0 / WS)

            # ---- working buffers ----
            qkv = wpool.tile([128, NT, 3 * HD], f16)      # natural qkv
            rs = wpool.tile([128, NT, 2], f32)            # rms scalars q,k
            qT_sb = wpool.tile([128, T], f16)             # q-hat.T [d, t]
            kT_sb = wpool.tile([128, NT, 128], f16)       # k-hat.T [d, si, s']
            pT_bufs = [wpool.tile([128, NT, TJ], f16, tag=f"pT{i}",
                                  name=f"pT{i}") for i in range(2)]

            # ============ phase 1: qkv (fp8 DoubleRow) + rms + rotary ======
            with tc.tile_pool(name="ps_qkv", bufs=6, space="PSUM") as pq, \
                 tc.tile_pool(name="ps_tr", bufs=2, space="PSUM") as ptr, \
                 tc.tile_pool(name="sq", bufs=2) as sqpool:

                def emit_qkv_group(g):
                    gs = slice(4 * g, 4 * (g + 1))
                    sq_g = sqpool.tile([128, 4, 2 * HD], f32, tag="sqg")
                    xh_b = xpool.tile([128, 4, NCH, 128], fp8,
                                      tag=f"xh{g % 2}", name=f"xh_b{g}")
                    xl_b = xpool.tile([128, 4, NCH, 128], fp8,
                                      tag=f"xl{g % 2}", name=f"xl_b{g}")
                    nc.sync.dma_start(xh_b[:], xh_d[g])
                    nc.sync.dma_start(xl_b[:], xl_d[g])
                    # consts ride the slack between x batches (x is critical)
                    if g == 0:
                        nc.sync.dma_start(cos_sb[:], cos_d[:])
                        nc.sync.dma_start(sin_sb[:], sin_d[:])
                    elif g == 1:
                        nc.sync.dma_start(idn_sb[:], idn_d[:])
                        nc.sync.dma_start(ve_sb[:], ve_d[:])
                    elif g == 2:
                        nc.sync.dma_start(tri_sb[:], tri_d[:])
                    else:
                        nc.sync.dma_start(cw_sb[:], cw_d[:])
                    for j in range(4):
                        ti = 4 * g + j
                        ps = pq.tile([128, 3 * HD], f32, tag="qkvp")
                        n_mm = NPAIR * 3 * 2
                        i_mm = 0
                        for (xa, wa) in ((xh_b, wh_sb), (xl_b, wh_sb),
                                         (xh_b, wl_sb)):
                            for P in range(NPAIR):
                                cs = slice(2 * P, 2 * P + 2)
                                for half in range(2):
                                    hs2 = slice(192 * half, 192 * (half + 1))
                                    nc.tensor.matmul(
                                        ps[:, hs2],
                                        xa[:, j, cs, :],
                                        wa[:, cs, hs2],
                                        start=(i_mm == 0),
                                        stop=(i_mm == n_mm - 1),
                                        perf_mode=DR,
                                    )
                                    i_mm += 1
                        # squares of q,k on ACT (reads PSUM, descale 1/WS)
                        nc.scalar.activation(sq_g[:, j, :], ps[:, 0:2 * HD],
                                             AF.Square, bias=zero_sb[:],
                                             scale=1.0 / WS)
                        # drain qkv tile to SBUF f16 with 1/WS descale
                        if ti % 2 == 0:
                            nc.scalar.activation(qkv[:, ti, :], ps[:], AF.Copy,
                                                 bias=0.0, scale=1.0 / WS)
                        else:
                            nc.vector.tensor_scalar_mul(qkv[:, ti, :], ps[:],
                                                        sc64_sb[:])
                    # per-group sumsq reduce: [128,4,256]->[128,8,128]->X
                    nc.vector.tensor_reduce(
                        rs[:, gs, :].rearrange("p a b -> p (a b)"),
                        sq_g.rearrange("p a (two d) -> p (a two) d", two=2),
                        axis=mybir.AxisListType.X,
                        op=ALU.add,
                    )
                    # rq = SCALE/sqrt(ms+eps), rk = 1/sqrt(ms+eps)
                    nc.scalar.activation(rs[:, gs, 0], rs[:, gs, 0], AF.Sqrt,
                                         scale=1.0 / (HD * SCALE * SCALE),
                                         bias=bq_sb[:])
                    nc.scalar.activation(rs[:, gs, 1], rs[:, gs, 1], AF.Sqrt,
                                         scale=1.0 / HD, bias=bk_sb[:])
                    nc.vector.reciprocal(rs[:, gs, :], rs[:, gs, :])

                def emit_tr_group(g):
                    hs = slice(4 * g, 4 * (g + 1))
                    for base in (0, HD):
                        eng = nc.vector if base == 0 else nc.gpsimd
                        x1 = qkv[:, hs, base + 0:base + 32]
                        x2 = qkv[:, hs, base + 64:base + 96]
                        t1 = wpool.tile([128, 4, 32], f16, tag=f"rot1{base}",
                                        name=f"t1_{base}")
                        t2 = wpool.tile([128, 4, 32], f16, tag=f"rot2{base}",
                                        name=f"t2_{base}")
                        t3 = wpool.tile([128, 4, 32], f16, tag=f"rot3{base}",
                                        name=f"t3_{base}")
                        t4 = wpool.tile([128, 4, 32], f16, tag=f"rot4{base}",
                                        name=f"t4_{base}")
                        eng.tensor_mul(t1[:], x1, cos_sb[:, hs, :])
                        eng.tensor_mul(t2[:], x2, sin_sb[:, hs, :])
                        eng.tensor_mul(t3[:], x2, cos_sb[:, hs, :])
                        eng.tensor_mul(t4[:], x1, sin_sb[:, hs, :])
                        eng.tensor_add(x1, t1[:], t2[:])
                        eng.tensor_sub(x2, t3[:], t4[:])
                    # diag(rq), diag(rk) for fused transpose+normalize
                    gs = slice(4 * g, 4 * (g + 1))
                    dq_g = wpool.tile([128, 4, 128], f16, tag=f"dq{g % 2}",
                                      name=f"dq_{g}")
                    dk_g = wpool.tile([128, 4, 128], f16, tag=f"dk{g % 2}",
                                      name=f"dk_{g}")
                    idn_b = _bcast_mid(idn_sb[:], 4)
                    nc.gpsimd.tensor_tensor(
                        dq_g[:], idn_b, _bcast(rs[:, gs, 0:1], HD),
                        op=ALU.mult)
                    nc.gpsimd.tensor_tensor(
                        dk_g[:], idn_b, _bcast(rs[:, gs, 1:2], HD),
                        op=ALU.mult)
                    # v-hat in place: v += ve_pre (ve prescaled by lambda1)
                    nc.vector.tensor_add(qkv[:, gs, 2 * HD:],
                                         qkv[:, gs, 2 * HD:], ve_sb[:, gs, :])
                    # transpose+normalize: out = qtile.T @ diag(r) (PE)
                    for base, isq in ((0, True), (HD, False)):
                        tp = ptr.tile([128, 4, 128], f32, tag="trp")
                        dmat = dq_g if isq else dk_g
                        for j in range(4):
                            ti = 4 * g + j
                            nc.tensor.matmul(
                                tp[:, j, :],
                                qkv[:, ti, base:base + HD],
                                dmat[:, j, :], start=True, stop=True)
                        dview = qT_sb[:, 512 * g:512 * (g + 1)] if isq \
                            else kT_sb[:, 4 * g:4 * (g + 1), :] \
                            .rearrange("p a b -> p (a b)")
                        tpf = tp[:].rearrange("p a b -> p (a b)")
                        if g % 2 == 0:
                            nc.vector.tensor_copy(dview, tpf)
                        else:
                            nc.scalar.copy(dview, tpf)

                # interleave: qkv(g) then transposes(g-1) keeps PE busy
                # while rotary/diag of g-1 complete on DVE/Pool
                for g in range(4):
                    emit_qkv_group(g)
                    if g >= 1:
                        emit_tr_group(g - 1)
                emit_tr_group(3)

            # ============ phase 2: attention + c_proj =====================
            # software-pipelined: scores(tj+1) are emitted before dn/yT(tj)
            # so the in-order PE queue never stalls on exp(tj).
            with tc.tile_pool(name="ps_sc", bufs=3, space="PSUM") as psc, \
                 tc.tile_pool(name="ps_dn", bufs=1, space="PSUM") as pdn, \
                 tc.tile_pool(name="ps_y", bufs=1, space="PSUM") as py, \
                 tc.tile_pool(name="ps_o", bufs=3, space="PSUM") as po, \
                 tc.tile_pool(name="fin", bufs=2) as fin, \
                 tc.tile_pool(name="stp", bufs=1) as stp:

                def emit_scores(tj):
                    pT_sb = pT_bufs[tj % 2]
                    t0 = TJ * tj
                    n_act = 4 * (tj + 1)
                    for si in range(n_act):
                        o = si - 4 * tj
                        off = 128 * o if o > 0 else 0
                        sc = psc.tile([128, TJ], f32, tag="scp")
                        nc.tensor.matmul(
                            sc[:, 0:TJ - off], kT_sb[:, si, :],
                            qT_sb[:, t0 + off:t0 + TJ],
                            start=True, stop=True)
                        nc.scalar.activation(
                            pT_sb[:, si, off:TJ], sc[:, 0:TJ - off], AF.Exp,
                            bias=bexp_sb[:])
                    # merged diagonal triangle mask: pT[4tj+o, 128o:128o+128]
                    import concourse.bass as bass
                    base = pT_sb[:, 4 * tj, 0:128]
                    mview = bass.AP(
                        tensor=base.tensor, offset=base.offset,
                        ap=[list(base.ap[0]), [TJ + 128, 4], [1, 128]])
                    tri_b = _bcast_mid(tri_sb[:], 4)
                    nc.vector.tensor_tensor(mview, mview, tri_b, op=ALU.mult)

                def emit_av(tj):
                    pT_sb = pT_bufs[tj % 2]
                    t0 = TJ * tj
                    n_act = 4 * (tj + 1)
                    dn = pdn.tile([128, TJ], f32, tag="dn")
                    yT = py.tile([128, TJ], f32, tag="yT")
                    for si in range(n_act):
                        o = si - 4 * tj
                        off = 128 * o if o > 0 else 0
                        first = (si == 0)
                        last = (si == n_act - 1)
                        nc.tensor.matmul(dn[:, off:TJ], ones_sb[:],
                                         pT_sb[:, si, off:TJ],
                                         start=first, stop=last)
                        nc.tensor.matmul(yT[:, off:TJ],
                                         qkv[:, si, 2 * HD:],
                                         pT_sb[:, si, off:TJ],
                                         start=first, stop=last)
                    rdn = fin.tile([128, TJ], f32, tag=f"rdn{tj % 2}", name=f"rdn_{tj}")
                    nc.vector.reciprocal(rdn[:], dn[:])
                    yh = fin.tile([128, TJ], f16, tag=f"yh{tj % 2}",
                                  name=f"yh_{tj}")
                    nc.vector.tensor_mul(yh[:], yT[:], rdn[:])
                    return yh

                def emit_cproj(tj, yh):
                    t0 = TJ * tj
                    st = stp.tile([128, 8, TJ], f16, tag=f"st{tj % 2}",
                                  name=f"st_{tj}")
                    for e in range(8):
                        pot = po.tile([128, TJ], f32, tag="pot")
                        nc.tensor.matmul(pot[:],
                                         cw_sb[:, 128 * e:128 * (e + 1)],
                                         yh[:],
                                         start=True, stop=True)
                        if e % 2 == 0:
                            nc.scalar.copy(st[:, e, :], pot[:])
                        else:
                            nc.vector.tensor_copy(st[:, e, :], pot[:])
                    # one batched store: [128, 8, TJ] -> out rows [1024, t-blk]
                    import concourse.bass as bass
                    dst = out_d[0:128, t0:t0 + TJ]
                    dstb = bass.AP(
                        tensor=dst.tensor, offset=dst.offset,
                        ap=[list(dst.ap[0]), [128 * T, 8], list(dst.ap[1])])
                    nc.sync.dma_start(dstb, st[:])

                # pipeline: scores two blocks ahead; av/cproj of tj separated
                # by other PE work so the DVE recip/mul chain is hidden
                emit_scores(0)
                emit_scores(1)
                yh0 = emit_av(0)
                emit_scores(2)
                emit_cproj(0, yh0)
                yh1 = emit_av(1)
                emit_scores(3)
                emit_cproj(1, yh1)
                yh2 = emit_av(2)
                emit_cproj(2, yh2)
                yh3 = emit_av(3)
                emit_cproj(3, yh3)

    nc.compile()
    _CACHE["nc"] = nc
    return nc


def _host_inputs(x, ve, qkv_w, lambdas, c_proj_w):
    """Build the 8 per-core input maps (layout transforms only)."""
    import ml_dtypes
    f16 = ml_dtypes.float16 if hasattr(ml_dtypes, "float16") else np.float16
    e4 = ml_dtypes.float8_e4m3
    x = np.asarray(x, np.float32)
    ve = np.asarray(ve, np.float32)
    qkv_w = np.asarray(qkv_w, np.float32)
    lambdas = np.asarray(lambdas, np.float32)
    c_proj_w = np.asarray(c_proj_w, np.float32)

    # x planes: [4 batch, 128 cin, 4 tile, NCH, 128 tok]
    xr = x[0].reshape(NT, 128, NCH, 128).transpose(0, 3, 2, 1)
    xr = xr.reshape(4, 4, 128, NCH, 128).transpose(0, 2, 1, 3, 4)
    xh = xr.astype(e4)
    xl = (xr - xh.astype(np.float32)).astype(e4)
    xh = np.ascontiguousarray(xh)
    xl = np.ascontiguousarray(xl)

    freq = (1.0 / 1024.0) ** np.linspace(0.0, 1.0, HD // 4, dtype=np.float32)
    theta = np.arange(T, dtype=np.float32)[:, None] * freq[None, :]  # [T, 32]
    cosT = np.ascontiguousarray(
        np.cos(theta).astype(f16).reshape(NT, 128, 32).transpose(1, 0, 2))
    sinT = np.ascontiguousarray(
        np.sin(theta).astype(f16).reshape(NT, 128, 32).transpose(1, 0, 2))
    tri = (np.arange(128)[None, :] >= np.arange(128)[:, None]).astype(f16)
    idn = np.eye(128, dtype=np.float32).astype(f16)

    lam0, lam1 = float(lambdas[0]), float(lambdas[1])
    wscale = np.concatenate([np.full(2 * HD, WS, np.float32),
                             np.full(HD, WS * lam0, np.float32)])

    in_maps = []
    for h in range(NCORES):
        sl = slice(128 * h, 128 * (h + 1))
        # W planes: [128 cin, NCH, 3*HD], prescaled
        whd = qkv_w[:, sl, :]                          # [3, 128, 1024]
        wt = whd.transpose(2, 0, 1).reshape(D, 3 * HD)  # [cin-full, 384]
        wt = wt * wscale[None, :]
        wt = wt.reshape(NCH, 128, 3 * HD).transpose(1, 0, 2)  # [128, NCH, 384]
        wh = wt.astype(e4)
        wl = (wt - wh.astype(np.float32)).astype(e4)
        wh = np.ascontiguousarray(wh)
        wl = np.ascontiguousarray(wl)
        veh = np.ascontiguousarray(
            (lam1 * ve[0, :, sl]).reshape(NT, 128, HD)
            .transpose(1, 0, 2).astype(f16))
        cwh = np.ascontiguousarray(c_proj_w[:, sl].T.astype(f16))  # [128, 1024]
        in_maps.append({
            "xh": xh, "xl": xl, "wh": wh, "wl": wl, "veN": veh, "cwT": cwh,
            "cosT": cosT, "sinT": sinT, "tri": tri, "idn": idn,
        })
    return in_maps


def run(x, ve, qkv_w, lambdas, c_proj_w, trace=False):
    from concourse.bass_utils import run_bass_kernel_spmd

    nc = _build_program()
    in_maps = _host_inputs(x, ve, qkv_w, lambdas, c_proj_w)
    res = run_bass_kernel_spmd(
        nc, in_maps, core_ids=list(range(NCORES)), trace=trace)
    acc = np.zeros((D, T), np.float64)
    for r in res.results:
        acc += r["outT"].astype(np.float64)
    out = acc.astype(np.float32).T.reshape(B, T, D)
    return out, res


def kernel(x, ve, qkv_w, lambdas, c_proj_w):
    out, _ = run(x, ve, qkv_w, lambdas, c_proj_w, trace=False)
    return out



# revision 7
# speedup vs baseline: 1.0437x; 1.0437x over previous
"""Causal self-attention (B=1, T=2048, D=1024, H=8, hd=128) on 8 trn2 cores.

Sharding: tensor-parallel over heads -- one head per core. Each core computes
its head's qkv projection, rms-norm+rotary, causal attention, and the c_proj
partial product for its head; the host sums the 8 partial [D, T] outputs.

v2 restructure (from the 81.1us baseline), engine-balance driven:
  - ACT diet: ACT keeps only Exp (paired 2-bank PSUM score tiles -> 20 exps
    instead of 40), Sqrt (8 tiny), half the qkv drains, and a share of the
    c_proj drains. Everything else moved to DVE/Pool. A scheduling-only dep
    forces all Sqrts before the first Exp so exactly 2 act-table loads are
    emitted (was 5 x 1283ns with mid-phase thrash).
  - squares+reduce fused into DVE tensor_tensor_reduce (accum_out) per tile;
    v-drain fused with the ve add via scalar_tensor_tensor.
  - diag(rq)/diag(rk) builds on DVE (f16 2x mode) instead of Pool.
  - DMA: first qkv group is term-major (all (xh,wh) matmuls for 4 tiles, then
    (xl,wh), then (xh,wl)) so PE starts right after xh+wh land (~4.5us);
    consts packed into one blob DMA; load order xh0,wh,xl0,wl,ve,cst,...
  - c_proj drains round-robin ACT/DVE/Pool; c_proj PE matmuls interleaved
    with score/av matmuls so the in-order PE queue never stalls on the
    2-buf PSUM drain; output stored in half-blocks (quarters for the last
    t-block) to shrink the tail.
"""

import numpy as np

B, T, D = 1, 2048, 1024
H, HD = 8, 128
SCALE = 0.12
NCORES = 8
NT = T // 128      # 16 token tiles
NCH = D // 128     # 8 contraction chunks
NPAIR = NCH // 2   # 4 DoubleRow chunk pairs
NTJ = 4            # attention t-blocks
TJ = T // NTJ      # 512
EPS = float(np.finfo(np.float32).eps)
WS = 64.0          # fp8 weight prescale
ESH = 4.5          # exp shift

_CACHE = {}


def _bcast(ap, n):
    """Broadcast a [..., 1] AP to [..., n] via a step-0 trailing dim."""
    try:
        return ap.to_broadcast(list(ap.shape[:-1]) + [n])
    except Exception:
        import concourse.bass as bass
        return bass.AP(tensor=ap.tensor, offset=ap.offset,
                       ap=list(ap.ap[:-1]) + [[0, n]])


def _bcast_mid(ap, n):
    """Insert a step-0 middle dim: [p, f] -> [p, n, f]."""
    import concourse.bass as bass
    return bass.AP(tensor=ap.tensor, offset=ap.offset,
                   ap=[list(ap.ap[0]), [0, n], list(ap.ap[1])])


def _view(tile_ap, start, dims):
    """View into a flat [128, N] AP at elem offset `start` with free dims
    [(stride, count), ...]."""
    import concourse.bass as bass
    return bass.AP(tensor=tile_ap.tensor, offset=tile_ap.offset + start,
                   ap=[list(tile_ap.ap[0])] + [list(d) for d in dims])


def _drive(*gens):
    """Round-robin drive generators to completion (interleaves PE work)."""
    gens = [g for g in gens if g is not None]
    while gens:
        nxt = []
        for g in gens:
            try:
                next(g)
                nxt.append(g)
            except StopIteration:
                pass
        gens = nxt


def _build_program():
    if "nc" in _CACHE:
        return _CACHE["nc"]

    import concourse.bacc as bacc
    import concourse.tile as tile
    import concourse.mybir as mybir
    from concourse.tile_rust import add_dep_helper

    f32 = mybir.dt.float32
    f16 = mybir.dt.float16
    fp8 = mybir.dt.float8e4
    AF = mybir.ActivationFunctionType
    ALU = mybir.AluOpType
    DR = mybir.MatmulPerfMode.DoubleRow

    nc = bacc.Bacc("TRN2", target_bir_lowering=False, debug=False)

    # x planes: [batch of 4 tiles, c-in-part 128, tile, chunk, token]
    xh_d = nc.dram_tensor("xh", [4, 128, 4, NCH, 128], fp8,
                          kind="ExternalInput")
    xl_d = nc.dram_tensor("xl", [4, 128, 4, NCH, 128], fp8,
                          kind="ExternalInput")
    wh_d = nc.dram_tensor("wh", [128, NCH, 3 * HD], fp8, kind="ExternalInput")
    wl_d = nc.dram_tensor("wl", [128, NCH, 3 * HD], fp8, kind="ExternalInput")
    ve_d = nc.dram_tensor("veN", [128, NT, HD], f16, kind="ExternalInput")
    cw_d = nc.dram_tensor("cwT", [HD, D], f16, kind="ExternalInput")
    # packed consts per partition: cos(512) | sin(512) | tri(128) | idn(128)
    cst_d = nc.dram_tensor("cst", [128, 1280], f16, kind="ExternalInput")
    out_d = nc.dram_tensor("outT", [D, T], f16, kind="ExternalOutput")

    with tile.TileContext(nc) as tc:
        with tc.tile_pool(name="const", bufs=1) as cpool, \
             tc.tile_pool(name="work", bufs=1) as wpool, \
             tc.tile_pool(name="xs", bufs=1) as xpool:
            # ---- resident inputs ----
            wh_sb = cpool.tile([128, NCH, 3 * HD], fp8)
            wl_sb = cpool.tile([128, NCH, 3 * HD], fp8)
            ve_sb = cpool.tile([128, NT, HD], f16)
            cw_sb = cpool.tile([HD, D], f16)
            cst_sb = cpool.tile([128, 1280], f16)
            ones_sb = cpool.tile([128, 128], f16)
            bexp_sb = cpool.tile([128, 1], f32)   # exp shift bias
            bq_sb = cpool.tile([128, 1], f32)     # eps/SCALE^2 bias for rq
            bk_sb = cpool.tile([128, 1], f32)     # eps bias for rk

            cstf = cst_sb[:]

            def cos_g(g):     # [128, 4, 32] for tile group g
                return _view(cstf, 128 * g, [[32, 4], [1, 32]])

            def sin_g(g):
                return _view(cstf, 512 + 128 * g, [[32, 4], [1, 32]])

            tri_v = _view(cstf, 1024, [[1, 128]])
            idn_v = _view(cstf, 1152, [[1, 128]])

            nc.vector.memset(ones_sb[:], 1.0)
            nc.vector.memset(bexp_sb[:], -ESH)
            nc.vector.memset(bq_sb[:], EPS / (SCALE * SCALE))
            nc.vector.memset(bk_sb[:], EPS)

            # ---- working buffers ----
            qkv = wpool.tile([128, NT, 3 * HD], f16)      # natural qkv
            rs = wpool.tile([128, NT, 2], f32)            # rms scalars q,k
            sqs = wpool.tile([128, HD], f16)              # TTR scratch out
            qT_sb = wpool.tile([128, T], f16)             # q-hat.T [d, t]
            kT_sb = wpool.tile([128, NT, 128], f16)       # k-hat.T [d, si, s']
            pT_bufs = [wpool.tile([128, NT, TJ], f16, tag=f"pT{i}",
                                  name=f"pT{i}") for i in range(2)]

            sqrt_insts = []   # for the act-table ordering dep
            exp_insts = []

            # ============ phase 1: qkv (fp8 DoubleRow) + rms + rotary ======
            with tc.tile_pool(name="ps_qkv", bufs=6, space="PSUM") as pq, \
                 tc.tile_pool(name="ps_tr", bufs=2, space="PSUM") as ptr:

                def dma_order_for_group(g):
                    # interleave remaining loads so x batches stay critical
                    if g == 1:
                        nc.sync.dma_start(cst_sb[:], cst_d[:])
                    elif g == 2:
                        nc.sync.dma_start(cw_sb[:], cw_d[:])

                def emit_qkv_mms(g, ps_tiles, xh_b, xl_b, term_major):
                    """24 DR matmuls per tile; term-major runs the 3 planes
                    as waves across all 4 tiles (x/w DMA arrival order)."""
                    terms = ((xh_b, wh_sb), (xl_b, wh_sb), (xh_b, wl_sb))
                    i_mm = [0] * 4
                    n_mm = NPAIR * 3 * 2

                    def one(j, t_idx, P, half):
                        xa, wa = terms[t_idx]
                        cs = slice(2 * P, 2 * P + 2)
                        hs2 = slice(192 * half, 192 * (half + 1))
                        nc.tensor.matmul(
                            ps_tiles[j][:, hs2],
                            xa[:, j, cs, :],
                            wa[:, cs, hs2],
                            start=(i_mm[j] == 0),
                            stop=(i_mm[j] == n_mm - 1),
                            perf_mode=DR,
                        )
                        i_mm[j] += 1

                    if term_major:
                        for t_idx in range(3):
                            for j in range(4):
                                for P in range(NPAIR):
                                    for half in range(2):
                                        one(j, t_idx, P, half)
                    else:
                        for j in range(4):
                            for t_idx in range(3):
                                for P in range(NPAIR):
                                    for half in range(2):
                                        one(j, t_idx, P, half)

                def emit_qkv_group(g):
                    gs = slice(4 * g, 4 * (g + 1))
                    xh_b = xpool.tile([128, 4, NCH, 128], fp8,
                                      tag=f"xh{g % 2}", name=f"xh_b{g}")
                    xl_b = xpool.tile([128, 4, NCH, 128], fp8,
                                      tag=f"xl{g % 2}", name=f"xl_b{g}")
                    if g == 0:
                        # critical order: first matmul wave needs xh+wh only
                        nc.sync.dma_start(xh_b[:], xh_d[g])
                        nc.sync.dma_start(wh_sb[:], wh_d[:])
                        nc.sync.dma_start(xl_b[:], xl_d[g])
                        nc.sync.dma_start(wl_sb[:], wl_d[:])
                        nc.sync.dma_start(ve_sb[:], ve_d[:])
                    else:
                        nc.sync.dma_start(xh_b[:], xh_d[g])
                        nc.sync.dma_start(xl_b[:], xl_d[g])
                    ps_tiles = [pq.tile([128, 3 * HD], f32, tag="qkvp",
                                        name=f"ps{g}_{j}") for j in range(4)]
                    emit_qkv_mms(g, ps_tiles, xh_b, xl_b, term_major=(g == 0))
                    dma_order_for_group(g)
                    for j in range(4):
                        ti = 4 * g + j
                        ps = ps_tiles[j]
                        # qk drain f32->f16 with 1/WS descale: ACT/DVE split
                        if ti % 2 == 0:
                            nc.scalar.activation(qkv[:, ti, 0:2 * HD],
                                                 ps[:, 0:2 * HD], AF.Copy,
                                                 bias=0.0, scale=1.0 / WS)
                        else:
                            nc.vector.tensor_scalar_mul(qkv[:, ti, 0:2 * HD],
                                                        ps[:, 0:2 * HD],
                                                        1.0 / WS)
                        # fused sumsq: rs[:,ti,c] = sum(q*q) via TTR accum
                        for cix, base in ((0, 0), (1, HD)):
                            nc.vector.tensor_tensor_reduce(
                                out=sqs[:],
                                in0=qkv[:, ti, base:base + HD],
                                in1=qkv[:, ti, base:base + HD],
                                scale=1.0,
                                scalar=0.0,
                                op0=ALU.mult,
                                op1=ALU.add,
                                accum_out=rs[:, ti, cix:cix + 1],
                            )
                    # v drain fused with ve add (ve prescaled by lambda1;
                    # lambda0 folded into W plane scale): v = ps_v/WS + ve
                    for j in range(4):
                        ti = 4 * g + j
                        nc.vector.scalar_tensor_tensor(
                            qkv[:, ti, 2 * HD:], ps_tiles[j][:, 2 * HD:],
                            1.0 / WS, ve_sb[:, ti, :],
                            op0=ALU.mult, op1=ALU.add)
                    # rq = SCALE/sqrt(ms+eps), rk = 1/sqrt(ms+eps)
                    i1 = nc.scalar.activation(rs[:, gs, 0], rs[:, gs, 0],
                                              AF.Sqrt,
                                              scale=1.0 / (HD * SCALE * SCALE),
                                              bias=bq_sb[:])
                    i2 = nc.scalar.activation(rs[:, gs, 1], rs[:, gs, 1],
                                              AF.Sqrt, scale=1.0 / HD,
                                              bias=bk_sb[:])
                    sqrt_insts.extend([i1, i2])
                    nc.vector.reciprocal(rs[:, gs, :], rs[:, gs, :])

                def emit_tr_group(g):
                    hs = slice(4 * g, 4 * (g + 1))
                    for base in (0, HD):
                        eng = nc.vector if base == 0 else nc.gpsimd
                        x1 = qkv[:, hs, base + 0:base + 32]
                        x2 = qkv[:, hs, base + 64:base + 96]
                        t1 = wpool.tile([128, 4, 32], f16, tag=f"rot1{base}",
                                        name=f"t1_{base}")
                        t2 = wpool.tile([128, 4, 32], f16, tag=f"rot2{base}",
                                        name=f"t2_{base}")
                        t3 = wpool.tile([128, 4, 32], f16, tag=f"rot3{base}",
                                        name=f"t3_{base}")
                        t4 = wpool.tile([128, 4, 32], f16, tag=f"rot4{base}",
                                        name=f"t4_{base}")
                        eng.tensor_mul(t1[:], x1, cos_g(g))
                        eng.tensor_mul(t2[:], x2, sin_g(g))
                        eng.tensor_mul(t3[:], x2, cos_g(g))
                        eng.tensor_mul(t4[:], x1, sin_g(g))
                        eng.tensor_add(x1, t1[:], t2[:])
                        eng.tensor_sub(x2, t3[:], t4[:])
                    # diag(rq), diag(rk) for fused transpose+normalize (DVE)
                    gs = slice(4 * g, 4 * (g + 1))
                    dq_g = wpool.tile([128, 4, 128], f16, tag=f"dq{g % 2}",
                                      name=f"dq_{g}")
                    dk_g = wpool.tile([128, 4, 128], f16, tag=f"dk{g % 2}",
                                      name=f"dk_{g}")
                    idn_b = _bcast_mid(idn_v, 4)
                    nc.vector.tensor_tensor(
                        dq_g[:], idn_b, _bcast(rs[:, gs, 0:1], HD),
                        op=ALU.mult)
                    nc.vector.tensor_tensor(
                        dk_g[:], idn_b, _bcast(rs[:, gs, 1:2], HD),
                        op=ALU.mult)
                    # transpose+normalize: out = qtile.T @ diag(r) (PE)
                    for base, isq in ((0, True), (HD, False)):
                        tp = ptr.tile([128, 4, 128], f32, tag="trp")
                        dmat = dq_g if isq else dk_g
                        for j in range(4):
                            ti = 4 * g + j
                            nc.tensor.matmul(
                                tp[:, j, :],
                                qkv[:, ti, base:base + HD],
                                dmat[:, j, :], start=True, stop=True)
                        dview = qT_sb[:, 512 * g:512 * (g + 1)] if isq \
                            else kT_sb[:, 4 * g:4 * (g + 1), :] \
                            .rearrange("p a b -> p (a b)")
                        tpf = tp[:].rearrange("p a b -> p (a b)")
                        # drains: alternate Pool / DVE
                        if isq == (g % 2 == 0):
                            nc.gpsimd.tensor_copy(dview, tpf)
                        else:
                            nc.vector.tensor_copy(dview, tpf)

                # interleave: qkv(g) then transposes(g-1) keeps PE busy
                for g in range(4):
                    emit_qkv_group(g)
                    if g >= 1:
                        emit_tr_group(g - 1)
                emit_tr_group(3)

            # ============ phase 2: attention + c_proj =====================
            with tc.tile_pool(name="ps_sc", bufs=2, space="PSUM") as psc, \
                 tc.tile_pool(name="ps_dn", bufs=1, space="PSUM") as pdn, \
                 tc.tile_pool(name="ps_y", bufs=1, space="PSUM") as py, \
                 tc.tile_pool(name="ps_o", bufs=2, space="PSUM") as po, \
                 tc.tile_pool(name="fin", bufs=2) as fin, \
                 tc.tile_pool(name="stp", bufs=1) as stp:

                def act_copy(dst, src):
                    nc.scalar.copy(dst, src)

                def dve_copy(dst, src):
                    nc.vector.tensor_copy(dst, src)

                def pool_copy(dst, src):
                    nc.gpsimd.tensor_copy(dst, src)

                drain_cycle = [act_copy, dve_copy, pool_copy, pool_copy,
                               act_copy, dve_copy, act_copy, dve_copy]

                def gen_scores(tj):
                    pT_sb = pT_bufs[tj % 2]
                    t0 = TJ * tj
                    n_act = 4 * (tj + 1)
                    for p in range(n_act // 2):
                        sc2 = psc.tile([128, 2, TJ], f32, tag="scp",
                                       name=f"sc2_{tj}_{p}")
                        offs = []
                        for b in range(2):
                            si = 2 * p + b
                            o = si - 4 * tj
                            off = 128 * o if o > 0 else 0
                            offs.append(off)
                            nc.tensor.matmul(
                                sc2[:, b, 0:TJ - off], kT_sb[:, si, :],
                                qT_sb[:, t0 + off:t0 + TJ],
                                start=True, stop=True)
                            yield
                        # paired exp over both banks (garbage regions of
                        # diagonal blocks are exp'd too but never read)
                        ei = nc.scalar.activation(
                            pT_sb[:, 2 * p:2 * p + 2, :], sc2[:], AF.Exp,
                            bias=bexp_sb[:])
                        exp_insts.append(ei)
                    # merged diagonal triangle mask (DVE)
                    import concourse.bass as bass
                    base = pT_sb[:, 4 * tj, 0:128]
                    mview = bass.AP(
                        tensor=base.tensor, offset=base.offset,
                        ap=[list(base.ap[0]), [TJ + 128, 4], [1, 128]])
                    tri_b = _bcast_mid(tri_v, 4)
                    nc.vector.tensor_tensor(mview, mview, tri_b, op=ALU.mult)

                def gen_av(tj, out_yh):
                    pT_sb = pT_bufs[tj % 2]
                    n_act = 4 * (tj + 1)
                    dn = pdn.tile([128, TJ], f32, tag="dn")
                    yT = py.tile([128, TJ], f32, tag="yT")
                    for si in range(n_act):
                        o = si - 4 * tj
                        off = 128 * o if o > 0 else 0
                        first = (si == 0)
                        last = (si == n_act - 1)
                        nc.tensor.matmul(dn[:, off:TJ], ones_sb[:],
                                         pT_sb[:, si, off:TJ],
                                         start=first, stop=last)
                        yield
                        nc.tensor.matmul(yT[:, off:TJ],
                                         qkv[:, si, 2 * HD:],
                                         pT_sb[:, si, off:TJ],
                                         start=first, stop=last)
                        yield
                    rdn = fin.tile([128, TJ], f32, tag=f"rdn{tj % 2}",
                                   name=f"rdn_{tj}")
                    nc.vector.reciprocal(rdn[:], dn[:])
                    yh = fin.tile([128, TJ], f16, tag=f"yh{tj % 2}",
                                  name=f"yh_{tj}")
                    nc.vector.tensor_mul(yh[:], yT[:], rdn[:])
                    out_yh.append(yh)

                def gen_cproj(tj, yh):
                    t0 = TJ * tj
                    st = stp.tile([128, 8, TJ], f16, tag=f"st{tj % 2}",
                                  name=f"st_{tj}")
                    import concourse.bass as bass
                    n_store = 4 if tj == 3 else 2   # split tail finer
                    e_per = 8 // n_store
                    for e in range(8):
                        pot = po.tile([128, TJ], f32, tag="pot")
                        nc.tensor.matmul(pot[:],
                                         cw_sb[:, 128 * e:128 * (e + 1)],
                                         yh[:],
                                         start=True, stop=True)
                        yield
                        drain_cycle[e](st[:, e, :], pot[:])
                        if e % e_per == e_per - 1:
                            e0 = e - e_per + 1
                            dst = out_d[0:128, t0:t0 + TJ]
                            dstb = bass.AP(
                                tensor=dst.tensor,
                                offset=dst.offset + 128 * e0 * T,
                                ap=[list(dst.ap[0]), [128 * T, e_per],
                                    list(dst.ap[1])])
                            nc.sync.dma_start(dstb, st[:, e0:e0 + e_per, :])

                # pipeline: scores two blocks ahead; cproj matmuls ride
                # between score/av matmuls so the PE queue never stalls on
                # the 2-buf cproj PSUM drains.
                yhs = []
                _drive(gen_scores(0))
                _drive(gen_scores(1))
                _drive(gen_av(0, yhs))
                _drive(gen_scores(2), gen_cproj(0, yhs[0]))
                _drive(gen_av(1, yhs))
                _drive(gen_scores(3), gen_cproj(1, yhs[1]))
                _drive(gen_av(2, yhs))
                _drive(gen_av(3, yhs), gen_cproj(2, yhs[2]))
                _drive(gen_cproj(3, yhs[3]))

            # act-table separation: all Sqrts precede the first Exp
            if sqrt_insts and exp_insts:
                add_dep_helper(exp_insts[0].ins, sqrt_insts[-1].ins,
                               sync=False,
                               reason="act-table: sqrt set before exp set")

    nc.compile()
    _CACHE["nc"] = nc
    return nc


def _host_inputs(x, ve, qkv_w, lambdas, c_proj_w):
    """Build the 8 per-core input maps (layout transforms only)."""
    import ml_dtypes
    f16 = ml_dtypes.float16 if hasattr(ml_dtypes, "float16") else np.float16
    e4 = ml_dtypes.float8_e4m3
    x = np.asarray(x, np.float32)
    ve = np.asarray(ve, np.float32)
    qkv_w = np.asarray(qkv_w, np.float32)
    lambdas = np.asarray(lambdas, np.float32)
    c_proj_w = np.asarray(c_proj_w, np.float32)

    # x planes: [4 batch, 128 cin, 4 tile, NCH, 128 tok]
    xr = x[0].reshape(NT, 128, NCH, 128).transpose(0, 3, 2, 1)
    xr = xr.reshape(4, 4, 128, NCH, 128).transpose(0, 2, 1, 3, 4)
    xh = xr.astype(e4)
    xl = (xr - xh.astype(np.float32)).astype(e4)
    xh = np.ascontiguousarray(xh)
    xl = np.ascontiguousarray(xl)

    freq = (1.0 / 1024.0) ** np.linspace(0.0, 1.0, HD // 4, dtype=np.float32)
    theta = np.arange(T, dtype=np.float32)[:, None] * freq[None, :]  # [T, 32]
    cosT = np.cos(theta).astype(f16).reshape(NT, 128, 32).transpose(1, 0, 2)
    sinT = np.sin(theta).astype(f16).reshape(NT, 128, 32).transpose(1, 0, 2)
    tri = (np.arange(128)[None, :] >= np.arange(128)[:, None]).astype(f16)
    idn = np.eye(128, dtype=np.float32).astype(f16)
    cst = np.ascontiguousarray(np.concatenate([
        cosT.reshape(128, 512), sinT.reshape(128, 512), tri, idn,
    ], axis=1))  # [128, 1280]

    lam0, lam1 = float(lambdas[0]), float(lambdas[1])
    wscale = np.concatenate([np.full(2 * HD, WS, np.float32),
                             np.full(HD, WS * lam0, np.float32)])

    in_maps = []
    for h in range(NCORES):
        sl = slice(128 * h, 128 * (h + 1))
        # W planes: [128 cin, NCH, 3*HD], prescaled
        whd = qkv_w[:, sl, :]                          # [3, 128, 1024]
        wt = whd.transpose(2, 0, 1).reshape(D, 3 * HD)  # [cin-full, 384]
        wt = wt * wscale[None, :]
        wt = wt.reshape(NCH, 128, 3 * HD).transpose(1, 0, 2)  # [128, NCH, 384]
        wh = wt.astype(e4)
        wl = (wt - wh.astype(np.float32)).astype(e4)
        wh = np.ascontiguousarray(wh)
        wl = np.ascontiguousarray(wl)
        veh = np.ascontiguousarray(
            (lam1 * ve[0, :, sl]).reshape(NT, 128, HD)
            .transpose(1, 0, 2).astype(f16))
        cwh = np.ascontiguousarray(c_proj_w[:, sl].T.astype(f16))  # [128, 1024]
        in_maps.append({
            "xh": xh, "xl": xl, "wh": wh, "wl": wl, "veN": veh, "cwT": cwh,
            "cst": cst,
        })
    return in_maps


def run(x, ve, qkv_w, lambdas, c_proj_w, trace=False):
    from concourse.bass_utils import run_bass_kernel_spmd

    nc = _build_program()
    in_maps = _host_inputs(x, ve, qkv_w, lambdas, c_proj_w)
    res = run_bass_kernel_spmd(
        nc, in_maps, core_ids=list(range(NCORES)), trace=trace)
    acc = np.zeros((D, T), np.float64)
    for r in res.results:
        acc += r["outT"].astype(np.float64)
    out = acc.astype(np.float32).T.reshape(B, T, D)
    return out, res


def kernel(x, ve, qkv_w, lambdas, c_proj_w):
    out, _ = run(x, ve, qkv_w, lambdas, c_proj_w, trace=False)
    return out


# revision 18
# speedup vs baseline: 1.0559x; 1.0117x over previous
"""Causal self-attention (B=1, T=2048, D=1024, H=8, hd=128) on 8 trn2 cores.

Sharding: tensor-parallel over heads -- one head per core. Each core computes
its head's qkv projection, rms-norm+rotary, causal attention, and the c_proj
partial product for its head; the host sums the 8 partial [D, T] outputs.

v2 restructure (from the 81.1us baseline), engine-balance driven:
  - ACT diet: ACT keeps only Exp (paired 2-bank PSUM score tiles -> 20 exps
    instead of 40), Sqrt (8 tiny), half the qkv drains, and a share of the
    c_proj drains. Everything else moved to DVE/Pool. A scheduling-only dep
    forces all Sqrts before the first Exp so exactly 2 act-table loads are
    emitted (was 5 x 1283ns with mid-phase thrash).
  - squares+reduce fused into DVE tensor_tensor_reduce (accum_out) per tile;
    v-drain fused with the ve add via scalar_tensor_tensor.
  - diag(rq)/diag(rk) builds on DVE (f16 2x mode) instead of Pool.
  - DMA: first qkv group is term-major (all (xh,wh) matmuls for 4 tiles, then
    (xl,wh), then (xh,wl)) so PE starts right after xh+wh land (~4.5us);
    consts packed into one blob DMA; load order xh0,wh,xl0,wl,ve,cst,...
  - c_proj drains round-robin ACT/DVE/Pool; c_proj PE matmuls interleaved
    with score/av matmuls so the in-order PE queue never stalls on the
    2-buf PSUM drain; output stored in half-blocks (quarters for the last
    t-block) to shrink the tail.
"""

import numpy as np

B, T, D = 1, 2048, 1024
H, HD = 8, 128
SCALE = 0.12
NCORES = 8
NT = T // 128      # 16 token tiles
NCH = D // 128     # 8 contraction chunks
NPAIR = NCH // 2   # 4 DoubleRow chunk pairs
NTJ = 4            # attention t-blocks
TJ = T // NTJ      # 512
EPS = float(np.finfo(np.float32).eps)
WS = 64.0          # fp8 weight prescale
ESH = 4.5          # exp shift

_CACHE = {}


def _bcast(ap, n):
    """Broadcast a [..., 1] AP to [..., n] via a step-0 trailing dim."""
    try:
        return ap.to_broadcast(list(ap.shape[:-1]) + [n])
    except Exception:
        import concourse.bass as bass
        return bass.AP(tensor=ap.tensor, offset=ap.offset,
                       ap=list(ap.ap[:-1]) + [[0, n]])


def _bcast_mid(ap, n):
    """Insert a step-0 middle dim: [p, f] -> [p, n, f]."""
    import concourse.bass as bass
    return bass.AP(tensor=ap.tensor, offset=ap.offset,
                   ap=[list(ap.ap[0]), [0, n], list(ap.ap[1])])


def _view(tile_ap, start, dims):
    """View into a flat [128, N] AP at elem offset `start` with free dims
    [(stride, count), ...]."""
    import concourse.bass as bass
    return bass.AP(tensor=tile_ap.tensor, offset=tile_ap.offset + start,
                   ap=[list(tile_ap.ap[0])] + [list(d) for d in dims])


def _drive(*gens):
    """Round-robin drive generators to completion (interleaves PE work)."""
    gens = [g for g in gens if g is not None]
    while gens:
        nxt = []
        for g in gens:
            try:
                next(g)
                nxt.append(g)
            except StopIteration:
                pass
        gens = nxt


def _build_program():
    if "nc" in _CACHE:
        return _CACHE["nc"]

    import concourse.bacc as bacc
    import concourse.tile as tile
    import concourse.mybir as mybir
    from concourse.tile_rust import add_dep_helper

    f32 = mybir.dt.float32
    f16 = mybir.dt.float16
    fp8 = mybir.dt.float8e4
    AF = mybir.ActivationFunctionType
    ALU = mybir.AluOpType
    DR = mybir.MatmulPerfMode.DoubleRow

    nc = bacc.Bacc("TRN2", target_bir_lowering=False, debug=False)

    # x planes: [batch of 4 tiles, c-in-part 128, tile, chunk, token]
    xh_d = nc.dram_tensor("xh", [4, 128, 4, NCH, 128], fp8,
                          kind="ExternalInput")
    xl_d = nc.dram_tensor("xl", [4, 128, 4, NCH, 128], fp8,
                          kind="ExternalInput")
    wh_d = nc.dram_tensor("wh", [128, NCH, 3 * HD], fp8, kind="ExternalInput")
    wl_d = nc.dram_tensor("wl", [128, NCH, 3 * HD], fp8, kind="ExternalInput")
    ve_d = nc.dram_tensor("veN", [128, NT, HD], f16, kind="ExternalInput")
    cw_d = nc.dram_tensor("cwT", [HD, D], f16, kind="ExternalInput")
    # packed consts per partition: cos(512) | sin(512) | tri(128) | idn(128)
    cst_d = nc.dram_tensor("cst", [128, 1280], f16, kind="ExternalInput")
    out_d = nc.dram_tensor("outT", [D, T], f16, kind="ExternalOutput")

    with tile.TileContext(nc) as tc:
        with tc.tile_pool(name="const", bufs=1) as cpool, \
             tc.tile_pool(name="work", bufs=1) as wpool, \
             tc.tile_pool(name="xs", bufs=1) as xpool:
            # ---- resident inputs ----
            wh_sb = cpool.tile([128, NCH, 3 * HD], fp8)
            wl_sb = cpool.tile([128, NCH, 3 * HD], fp8)
            ve_sb = cpool.tile([128, NT, HD], f16)
            cw_sb = cpool.tile([HD, D], f16)
            cst_sb = cpool.tile([128, 1280], f16)
            ones_sb = cpool.tile([128, 128], f16)
            bexp_sb = cpool.tile([128, 1], f32)   # exp shift bias
            bq_sb = cpool.tile([128, 1], f32)     # eps/SCALE^2 bias for rq
            bk_sb = cpool.tile([128, 1], f32)     # eps bias for rk

            cstf = cst_sb[:]

            def cos_g(g):     # [128, 4, 32] for tile group g
                return _view(cstf, 128 * g, [[32, 4], [1, 32]])

            def sin_g(g):
                return _view(cstf, 512 + 128 * g, [[32, 4], [1, 32]])

            tri_v = _view(cstf, 1024, [[1, 128]])
            idn_v = _view(cstf, 1152, [[1, 128]])

            nc.vector.memset(ones_sb[:], 1.0)
            nc.vector.memset(bexp_sb[:], -ESH)
            nc.vector.memset(bq_sb[:], EPS / (SCALE * SCALE))
            nc.vector.memset(bk_sb[:], EPS)

            # ---- working buffers ----
            qkv = wpool.tile([128, NT, 3 * HD], f16)      # natural qkv
            rs = wpool.tile([128, NT, 2], f32)            # rms scalars q,k
            qT_sb = wpool.tile([128, T], f16)             # q-hat.T [d, t]
            kT_sb = wpool.tile([128, NT, 128], f16)       # k-hat.T [d, si, s']
            pT_bufs = [wpool.tile([128, NT, TJ], f16, tag=f"pT{i}",
                                  name=f"pT{i}") for i in range(2)]

            sqrt_insts = []   # for the act-table ordering dep
            exp_insts = []

            # ============ phase 1: qkv (fp8 DoubleRow) + rms + rotary ======
            with tc.tile_pool(name="ps_qkv", bufs=6, space="PSUM") as pq, \
                 tc.tile_pool(name="ps_tr", bufs=2, space="PSUM") as ptr:

                def dma_order_for_group(g):
                    # interleave remaining loads so x batches stay critical
                    if g == 2:
                        nc.sync.dma_start(cw_sb[:], cw_d[:])

                def emit_qkv_mms(g, ps_tiles, xh_b, xl_b, term_major):
                    """24 DR matmuls per tile; term-major runs the 3 planes
                    as waves across all 4 tiles (x/w DMA arrival order)."""
                    terms = ((xh_b, wh_sb), (xl_b, wh_sb), (xh_b, wl_sb))
                    i_mm = [0] * 4
                    n_mm = NPAIR * 3 * 2

                    def one(j, t_idx, P, half):
                        xa, wa = terms[t_idx]
                        cs = slice(2 * P, 2 * P + 2)
                        hs2 = slice(192 * half, 192 * (half + 1))
                        nc.tensor.matmul(
                            ps_tiles[j][:, hs2],
                            xa[:, j, cs, :],
                            wa[:, cs, hs2],
                            start=(i_mm[j] == 0),
                            stop=(i_mm[j] == n_mm - 1),
                            perf_mode=DR,
                        )
                        i_mm[j] += 1

                    if term_major:
                        for t_idx in range(3):
                            for j in range(4):
                                for P in range(NPAIR):
                                    for half in range(2):
                                        one(j, t_idx, P, half)
                    else:
                        for j in range(4):
                            for t_idx in range(3):
                                for P in range(NPAIR):
                                    for half in range(2):
                                        one(j, t_idx, P, half)

                def emit_qkv_group(g):
                    gs = slice(4 * g, 4 * (g + 1))
                    xh_b = xpool.tile([128, 4, NCH, 128], fp8,
                                      tag=f"xh{g % 2}", name=f"xh_b{g}")
                    xl_b = xpool.tile([128, 4, NCH, 128], fp8,
                                      tag=f"xl{g % 2}", name=f"xl_b{g}")
                    if g == 0:
                        # critical order: first matmul wave needs xh+wh only
                        nc.sync.dma_start(xh_b[:], xh_d[g])
                        nc.sync.dma_start(wh_sb[:], wh_d[:])
                        nc.sync.dma_start(xl_b[:], xl_d[g])
                        nc.sync.dma_start(wl_sb[:], wl_d[:])
                        nc.sync.dma_start(ve_sb[:], ve_d[:])
                        nc.sync.dma_start(cst_sb[:], cst_d[:])
                    else:
                        nc.sync.dma_start(xh_b[:], xh_d[g])
                        nc.sync.dma_start(xl_b[:], xl_d[g])
                    ps_tiles = [pq.tile([128, 3 * HD], f32, tag="qkvp",
                                        name=f"ps{g}_{j}") for j in range(4)]
                    emit_qkv_mms(g, ps_tiles, xh_b, xl_b, term_major=(g == 0))
                    dma_order_for_group(g)
                    sq_g = wpool.tile([128, 4, 2 * HD], f16, tag="sqg",
                                      name=f"sq_{g}")
                    for j in range(4):
                        ti = 4 * g + j
                        ps = ps_tiles[j]
                        # qk drain f32->f16 with 1/WS descale: ACT/DVE split
                        if ti % 2 == 0:
                            nc.scalar.activation(qkv[:, ti, 0:2 * HD],
                                                 ps[:, 0:2 * HD], AF.Copy,
                                                 bias=0.0, scale=1.0 / WS)
                        else:
                            nc.vector.tensor_scalar_mul(qkv[:, ti, 0:2 * HD],
                                                        ps[:, 0:2 * HD],
                                                        1.0 / WS)
                        # squares of q,k (f16, DVE 2x mode)
                        nc.vector.tensor_mul(sq_g[:, j, :],
                                             qkv[:, ti, 0:2 * HD],
                                             qkv[:, ti, 0:2 * HD])
                    # per-group sumsq reduce: [128,4,256]->[128,8,128]->X
                    nc.vector.tensor_reduce(
                        rs[:, gs, :].rearrange("p a b -> p (a b)"),
                        sq_g[:].rearrange("p a (two d) -> p (a two) d", two=2),
                        axis=mybir.AxisListType.X,
                        op=ALU.add,
                    )
                    # v drain fused with ve add (ve prescaled by lambda1;
                    # lambda0 folded into W plane scale): v = ps_v/WS + ve
                    for j in range(4):
                        ti = 4 * g + j
                        nc.vector.scalar_tensor_tensor(
                            qkv[:, ti, 2 * HD:], ps_tiles[j][:, 2 * HD:],
                            1.0 / WS, ve_sb[:, ti, :],
                            op0=ALU.mult, op1=ALU.add)
                    # rq = SCALE/sqrt(ms+eps), rk = 1/sqrt(ms+eps)
                    i1 = nc.scalar.activation(rs[:, gs, 0], rs[:, gs, 0],
                                              AF.Sqrt,
                                              scale=1.0 / (HD * SCALE * SCALE),
                                              bias=bq_sb[:])
                    i2 = nc.scalar.activation(rs[:, gs, 1], rs[:, gs, 1],
                                              AF.Sqrt, scale=1.0 / HD,
                                              bias=bk_sb[:])
                    sqrt_insts.extend([i1, i2])
                    nc.vector.reciprocal(rs[:, gs, :], rs[:, gs, :])

                def emit_tr_group(g):
                    hs = slice(4 * g, 4 * (g + 1))
                    for base in (0, HD):
                        eng = nc.vector if base == 0 else nc.gpsimd
                        x1 = qkv[:, hs, base + 0:base + 32]
                        x2 = qkv[:, hs, base + 64:base + 96]
                        t1 = wpool.tile([128, 4, 32], f16, tag=f"rot1{base}",
                                        name=f"t1_{base}")
                        t2 = wpool.tile([128, 4, 32], f16, tag=f"rot2{base}",
                                        name=f"t2_{base}")
                        t3 = wpool.tile([128, 4, 32], f16, tag=f"rot3{base}",
                                        name=f"t3_{base}")
                        t4 = wpool.tile([128, 4, 32], f16, tag=f"rot4{base}",
                                        name=f"t4_{base}")
                        eng.tensor_mul(t1[:], x1, cos_g(g))
                        eng.tensor_mul(t2[:], x2, sin_g(g))
                        eng.tensor_mul(t3[:], x2, cos_g(g))
                        eng.tensor_mul(t4[:], x1, sin_g(g))
                        eng.tensor_add(x1, t1[:], t2[:])
                        eng.tensor_sub(x2, t3[:], t4[:])
                    # normalize in place: q *= rq[t], k *= rk[t] (per-token
                    # scalar AP exempt from the DVE 2x-mode dtype check)
                    for j in range(4):
                        ti = 4 * g + j
                        nc.vector.tensor_scalar_mul(qkv[:, ti, 0:HD],
                                                    qkv[:, ti, 0:HD],
                                                    rs[:, ti, 0:1])
                        nc.vector.tensor_scalar_mul(qkv[:, ti, HD:2 * HD],
                                                    qkv[:, ti, HD:2 * HD],
                                                    rs[:, ti, 1:2])
                    # plain transposes (PE), drains on Pool
                    for base, isq in ((0, True), (HD, False)):
                        tp = ptr.tile([128, 4, 128], f32, tag="trp")
                        for j in range(4):
                            ti = 4 * g + j
                            nc.tensor.matmul(
                                tp[:, j, :],
                                qkv[:, ti, base:base + HD],
                                idn_v, start=True, stop=True)
                        dview = qT_sb[:, 512 * g:512 * (g + 1)] if isq \
                            else kT_sb[:, 4 * g:4 * (g + 1), :] \
                            .rearrange("p a b -> p (a b)")
                        tpf = tp[:].rearrange("p a b -> p (a b)")
                        nc.gpsimd.tensor_copy(dview, tpf)

                # interleave: qkv(g) then transposes(g-1) keeps PE busy
                for g in range(4):
                    emit_qkv_group(g)
                    if g >= 1:
                        emit_tr_group(g - 1)
                emit_tr_group(3)

            # ============ phase 2: attention + c_proj =====================
            with tc.tile_pool(name="ps_sc", bufs=2, space="PSUM") as psc, \
                 tc.tile_pool(name="ps_dn", bufs=1, space="PSUM") as pdn, \
                 tc.tile_pool(name="ps_y", bufs=1, space="PSUM") as py, \
                 tc.tile_pool(name="ps_o", bufs=2, space="PSUM") as po, \
                 tc.tile_pool(name="fin", bufs=2) as fin, \
                 tc.tile_pool(name="stp", bufs=1) as stp:

                def act_copy(dst, src):
                    nc.scalar.copy(dst, src)

                def dve_copy(dst, src):
                    nc.vector.tensor_copy(dst, src)

                def pool_copy(dst, src):
                    nc.gpsimd.tensor_copy(dst, src)

                drain_cycle = [act_copy, dve_copy, pool_copy, pool_copy,
                               act_copy, dve_copy, pool_copy, pool_copy]

                def gen_scores(tj):
                    pT_sb = pT_bufs[tj % 2]
                    t0 = TJ * tj
                    n_act = 4 * (tj + 1)
                    for p in range(n_act // 2):
                        sc2 = psc.tile([128, 2, TJ], f32, tag="scp",
                                       name=f"sc2_{tj}_{p}")
                        offs = []
                        for b in range(2):
                            si = 2 * p + b
                            o = si - 4 * tj
                            off = 128 * o if o > 0 else 0
                            offs.append(off)
                            nc.tensor.matmul(
                                sc2[:, b, 0:TJ - off], kT_sb[:, si, :],
                                qT_sb[:, t0 + off:t0 + TJ],
                                start=True, stop=True)
                            yield
                        # paired exp over both banks (garbage regions of
                        # diagonal blocks are exp'd too but never read)
                        ei = nc.scalar.activation(
                            pT_sb[:, 2 * p:2 * p + 2, :], sc2[:], AF.Exp,
                            bias=bexp_sb[:])
                        exp_insts.append(ei)
                    # merged diagonal triangle mask (DVE)
                    import concourse.bass as bass
                    base = pT_sb[:, 4 * tj, 0:128]
                    mview = bass.AP(
                        tensor=base.tensor, offset=base.offset,
                        ap=[list(base.ap[0]), [TJ + 128, 4], [1, 128]])
                    tri_b = _bcast_mid(tri_v, 4)
                    nc.vector.tensor_tensor(mview, mview, tri_b, op=ALU.mult)

                def gen_av(tj, out_yh):
                    pT_sb = pT_bufs[tj % 2]
                    n_act = 4 * (tj + 1)
                    dn = pdn.tile([128, TJ], f32, tag="dn")
                    yT = py.tile([128, TJ], f32, tag="yT")
                    for si in range(n_act):
                        o = si - 4 * tj
                        off = 128 * o if o > 0 else 0
                        first = (si == 0)
                        last = (si == n_act - 1)
                        nc.tensor.matmul(dn[:, off:TJ], ones_sb[:],
                                         pT_sb[:, si, off:TJ],
                                         start=first, stop=last)
                        yield
                        nc.tensor.matmul(yT[:, off:TJ],
                                         qkv[:, si, 2 * HD:],
                                         pT_sb[:, si, off:TJ],
                                         start=first, stop=last)
                        yield
                    rdn = fin.tile([128, TJ], f32, tag=f"rdn{tj % 2}",
                                   name=f"rdn_{tj}")
                    nc.vector.reciprocal(rdn[:], dn[:])
                    yh = fin.tile([128, TJ], f16, tag=f"yh{tj % 2}",
                                  name=f"yh_{tj}")
                    nc.vector.tensor_mul(yh[:], yT[:], rdn[:])
                    out_yh.append(yh)

                def gen_cproj(tj, yh):
                    t0 = TJ * tj
                    st = stp.tile([128, 8, TJ], f16, tag=f"st{tj % 2}",
                                  name=f"st_{tj}")
                    import concourse.bass as bass
                    e_per = 4
                    for e in range(8):
                        pot = po.tile([128, TJ], f32, tag="pot")
                        nc.tensor.matmul(pot[:],
                                         cw_sb[:, 128 * e:128 * (e + 1)],
                                         yh[:],
                                         start=True, stop=True)
                        yield
                        drain_cycle[e](st[:, e, :], pot[:])
                        if e % e_per == e_per - 1:
                            e0 = e - e_per + 1
                            dst = out_d[0:128, t0:t0 + TJ]
                            dstb = bass.AP(
                                tensor=dst.tensor,
                                offset=dst.offset + 128 * e0 * T,
                                ap=[list(dst.ap[0]), [128 * T, e_per],
                                    list(dst.ap[1])])
                            # Pool-issued: DMA_SEQ cost is ~25ns there vs
                            # 565+ on SP, so tail stores don't serialize
                            nc.gpsimd.dma_start(dstb, st[:, e0:e0 + e_per, :])

                # pipeline: scores two blocks ahead; cproj matmuls ride
                # between score/av matmuls so the PE queue never stalls on
                # the 2-buf cproj PSUM drains.
                yhs = []
                _drive(gen_scores(0))
                _drive(gen_scores(1))
                _drive(gen_av(0, yhs))
                _drive(gen_scores(2), gen_cproj(0, yhs[0]))
                _drive(gen_av(1, yhs))
                _drive(gen_scores(3), gen_cproj(1, yhs[1]))
                _drive(gen_av(2, yhs))
                _drive(gen_av(3, yhs), gen_cproj(2, yhs[2]))
                _drive(gen_cproj(3, yhs[3]))

            # act-table separation: all Sqrts precede the first Exp
            if sqrt_insts and exp_insts:
                add_dep_helper(exp_insts[0].ins, sqrt_insts[-1].ins,
                               sync=False,
                               reason="act-table: sqrt set before exp set")

    nc.compile()
    _CACHE["nc"] = nc
    return nc


def _host_inputs(x, ve, qkv_w, lambdas, c_proj_w):
    """Build the 8 per-core input maps (layout transforms only)."""
    import ml_dtypes
    f16 = ml_dtypes.float16 if hasattr(ml_dtypes, "float16") else np.float16
    e4 = ml_dtypes.float8_e4m3
    x = np.asarray(x, np.float32)
    ve = np.asarray(ve, np.float32)
    qkv_w = np.asarray(qkv_w, np.float32)
    lambdas = np.asarray(lambdas, np.float32)
    c_proj_w = np.asarray(c_proj_w, np.float32)

    # x planes: [4 batch, 128 cin, 4 tile, NCH, 128 tok]
    xr = x[0].reshape(NT, 128, NCH, 128).transpose(0, 3, 2, 1)
    xr = xr.reshape(4, 4, 128, NCH, 128).transpose(0, 2, 1, 3, 4)
    xh = xr.astype(e4)
    xl = (xr - xh.astype(np.float32)).astype(e4)
    xh = np.ascontiguousarray(xh)
    xl = np.ascontiguousarray(xl)

    freq = (1.0 / 1024.0) ** np.linspace(0.0, 1.0, HD // 4, dtype=np.float32)
    theta = np.arange(T, dtype=np.float32)[:, None] * freq[None, :]  # [T, 32]
    cosT = np.cos(theta).astype(f16).reshape(NT, 128, 32).transpose(1, 0, 2)
    sinT = np.sin(theta).astype(f16).reshape(NT, 128, 32).transpose(1, 0, 2)
    tri = (np.arange(128)[None, :] >= np.arange(128)[:, None]).astype(f16)
    idn = np.eye(128, dtype=np.float32).astype(f16)
    cst = np.ascontiguousarray(np.concatenate([
        cosT.reshape(128, 512), sinT.reshape(128, 512), tri, idn,
    ], axis=1))  # [128, 1280]

    lam0, lam1 = float(lambdas[0]), float(lambdas[1])
    wscale = np.concatenate([np.full(2 * HD, WS, np.float32),
                             np.full(HD, WS * lam0, np.float32)])

    in_maps = []
    for h in range(NCORES):
        sl = slice(128 * h, 128 * (h + 1))
        # W planes: [128 cin, NCH, 3*HD], prescaled
        whd = qkv_w[:, sl, :]                          # [3, 128, 1024]
        wt = whd.transpose(2, 0, 1).reshape(D, 3 * HD)  # [cin-full, 384]
        wt = wt * wscale[None, :]
        wt = wt.reshape(NCH, 128, 3 * HD).transpose(1, 0, 2)  # [128, NCH, 384]
        wh = wt.astype(e4)
        wl = (wt - wh.astype(np.float32)).astype(e4)
        wh = np.ascontiguousarray(wh)
        wl = np.ascontiguousarray(wl)
        veh = np.ascontiguousarray(
            (lam1 * ve[0, :, sl]).reshape(NT, 128, HD)
            .transpose(1, 0, 2).astype(f16))
        cwh = np.ascontiguousarray(c_proj_w[:, sl].T.astype(f16))  # [128, 1024]
        in_maps.append({
            "xh": xh, "xl": xl, "wh": wh, "wl": wl, "veN": veh, "cwT": cwh,
            "cst": cst,
        })
    return in_maps


def run(x, ve, qkv_w, lambdas, c_proj_w, trace=False):
    from concourse.bass_utils import run_bass_kernel_spmd

    nc = _build_program()
    in_maps = _host_inputs(x, ve, qkv_w, lambdas, c_proj_w)
    res = run_bass_kernel_spmd(
        nc, in_maps, core_ids=list(range(NCORES)), trace=trace)
    acc = np.zeros((D, T), np.float64)
    for r in res.results:
        acc += r["outT"].astype(np.float64)
    out = acc.astype(np.float32).T.reshape(B, T, D)
    return out, res


def kernel(x, ve, qkv_w, lambdas, c_proj_w):
    out, _ = run(x, ve, qkv_w, lambdas, c_proj_w, trace=False)
    return out


# revision 27
# speedup vs baseline: 1.0862x; 1.0287x over previous
"""Causal self-attention (B=1, T=2048, D=1024, H=8, hd=128) on 8 trn2 cores.

Sharding: tensor-parallel over heads -- one head per core. Each core computes
its head's qkv projection, rms-norm+rotary, causal attention, and the c_proj
partial product for its head; the host sums the 8 partial [D, T] outputs.

v2 restructure (from the 81.1us baseline), engine-balance driven:
  - ACT diet: ACT keeps only Exp (paired 2-bank PSUM score tiles -> 20 exps
    instead of 40), Sqrt (8 tiny), half the qkv drains, and a share of the
    c_proj drains. Everything else moved to DVE/Pool. A scheduling-only dep
    forces all Sqrts before the first Exp so exactly 2 act-table loads are
    emitted (was 5 x 1283ns with mid-phase thrash).
  - squares+reduce fused into DVE tensor_tensor_reduce (accum_out) per tile;
    v-drain fused with the ve add via scalar_tensor_tensor.
  - diag(rq)/diag(rk) builds on DVE (f16 2x mode) instead of Pool.
  - DMA: first qkv group is term-major (all (xh,wh) matmuls for 4 tiles, then
    (xl,wh), then (xh,wl)) so PE starts right after xh+wh land (~4.5us);
    consts packed into one blob DMA; load order xh0,wh,xl0,wl,ve,cst,...
  - c_proj drains round-robin ACT/DVE/Pool; c_proj PE matmuls interleaved
    with score/av matmuls so the in-order PE queue never stalls on the
    2-buf PSUM drain; output stored in half-blocks (quarters for the last
    t-block) to shrink the tail.
"""

import numpy as np

B, T, D = 1, 2048, 1024
H, HD = 8, 128
SCALE = 0.12
NCORES = 8
NT = T // 128      # 16 token tiles
NCH = D // 128     # 8 contraction chunks
NPAIR = NCH // 2   # 4 DoubleRow chunk pairs
NTJ = 4            # attention t-blocks
TJ = T // NTJ      # 512
EPS = float(np.finfo(np.float32).eps)
WS = 64.0          # fp8 weight prescale
ESH = 4.5          # exp shift

_CACHE = {}


def _bcast(ap, n):
    """Broadcast a [..., 1] AP to [..., n] via a step-0 trailing dim."""
    try:
        return ap.to_broadcast(list(ap.shape[:-1]) + [n])
    except Exception:
        import concourse.bass as bass
        return bass.AP(tensor=ap.tensor, offset=ap.offset,
                       ap=list(ap.ap[:-1]) + [[0, n]])


def _bcast_mid(ap, n):
    """Insert a step-0 middle dim: [p, f] -> [p, n, f]."""
    import concourse.bass as bass
    return bass.AP(tensor=ap.tensor, offset=ap.offset,
                   ap=[list(ap.ap[0]), [0, n], list(ap.ap[1])])


def _view(tile_ap, start, dims):
    """View into a flat [128, N] AP at elem offset `start` with free dims
    [(stride, count), ...]."""
    import concourse.bass as bass
    return bass.AP(tensor=tile_ap.tensor, offset=tile_ap.offset + start,
                   ap=[list(tile_ap.ap[0])] + [list(d) for d in dims])


def _drive(*gens):
    """Round-robin drive generators to completion (interleaves PE work)."""
    gens = [g for g in gens if g is not None]
    while gens:
        nxt = []
        for g in gens:
            try:
                next(g)
                nxt.append(g)
            except StopIteration:
                pass
        gens = nxt


def _build_program():
    if "nc" in _CACHE:
        return _CACHE["nc"]

    import concourse.bacc as bacc
    import concourse.tile as tile
    import concourse.mybir as mybir
    from concourse.tile_rust import add_dep_helper

    f32 = mybir.dt.float32
    f16 = mybir.dt.float16
    fp8 = mybir.dt.float8e4
    AF = mybir.ActivationFunctionType
    ALU = mybir.AluOpType
    DR = mybir.MatmulPerfMode.DoubleRow

    nc = bacc.Bacc("TRN2", target_bir_lowering=False, debug=False)

    # x planes: [batch of 4 tiles, c-in-part 128, tile, chunk, token]
    xh_d = nc.dram_tensor("xh", [4, 128, 4, NCH, 128], fp8,
                          kind="ExternalInput")
    xl_d = nc.dram_tensor("xl", [4, 128, 4, NCH, 128], fp8,
                          kind="ExternalInput")
    wh_d = nc.dram_tensor("wh", [128, NCH, 3 * HD], fp8, kind="ExternalInput")
    wl_d = nc.dram_tensor("wl", [128, NCH, 3 * HD], fp8, kind="ExternalInput")
    ve_d = nc.dram_tensor("veN", [128, NT, HD], f16, kind="ExternalInput")
    cw_d = nc.dram_tensor("cwT", [HD, D], f16, kind="ExternalInput")
    # packed consts per partition: cos(512) | sin(512) | tri(128) | idn(128)
    cst_d = nc.dram_tensor("cst", [128, 1280], f16, kind="ExternalInput")
    out_d = nc.dram_tensor("outT", [D, T], f16, kind="ExternalOutput")

    with tile.TileContext(nc) as tc:
        with tc.tile_pool(name="const", bufs=1) as cpool, \
             tc.tile_pool(name="work", bufs=1) as wpool, \
             tc.tile_pool(name="xs", bufs=1) as xpool:
            # ---- resident inputs ----
            wh_sb = cpool.tile([128, NCH, 3 * HD], fp8)
            wl_sb = cpool.tile([128, NCH, 3 * HD], fp8)
            ve_sb = cpool.tile([128, NT, HD], f16)
            cw_sb = cpool.tile([HD, D], f16)
            cst_sb = cpool.tile([128, 1280], f16)
            ones_sb = cpool.tile([128, 128], f16)
            bexp_sb = cpool.tile([128, 1], f32)   # exp shift bias

            cstf = cst_sb[:]

            def cos_g(g):     # [128, 4, 32] for tile group g
                return _view(cstf, 128 * g, [[32, 4], [1, 32]])

            def sin_g(g):
                return _view(cstf, 512 + 128 * g, [[32, 4], [1, 32]])

            tri_v = _view(cstf, 1024, [[1, 128]])
            idn_v = _view(cstf, 1152, [[1, 128]])

            nc.vector.memset(ones_sb[:], 1.0)
            nc.vector.memset(bexp_sb[:], -ESH)

            # ---- working buffers ----
            qkv = wpool.tile([128, NT, 3 * HD], f16)      # natural qkv
            rs = wpool.tile([128, NT, 2], f32)            # rms scalars q,k
            rx = wpool.tile([128, NT, 2], f32)            # rsqrt arg scratch
            rt = wpool.tile([128, NT, 2], f32)            # newton scratch
            qT_sb = wpool.tile([128, T], f16)             # q-hat.T [d, t]
            kT_sb = wpool.tile([128, NT, 128], f16)       # k-hat.T [d, si, s']
            pT_bufs = [wpool.tile([128, NT, TJ], f16, tag=f"pT{i}",
                                  name=f"pT{i}") for i in range(2)]

            # ============ phase 1: qkv (fp8 DoubleRow) + rms + rotary ======
            with tc.tile_pool(name="ps_qkv", bufs=6, space="PSUM") as pq, \
                 tc.tile_pool(name="ps_tr", bufs=2, space="PSUM") as ptr:

                def dma_order_for_group(g):
                    # interleave remaining loads so x batches stay critical
                    if g == 2:
                        nc.sync.dma_start(cw_sb[:], cw_d[:])

                def emit_qkv_mms(g, ps_tiles, xh_b, xl_b, term_major):
                    """24 DR matmuls per tile; term-major runs the 3 planes
                    as waves across all 4 tiles (x/w DMA arrival order)."""
                    terms = ((xh_b, wh_sb), (xl_b, wh_sb), (xh_b, wl_sb))
                    i_mm = [0] * 4
                    n_mm = NPAIR * 3 * 2

                    def one(j, t_idx, P, half):
                        xa, wa = terms[t_idx]
                        cs = slice(2 * P, 2 * P + 2)
                        hs2 = slice(192 * half, 192 * (half + 1))
                        nc.tensor.matmul(
                            ps_tiles[j][:, hs2],
                            xa[:, j, cs, :],
                            wa[:, cs, hs2],
                            start=(i_mm[j] == 0),
                            stop=(i_mm[j] == n_mm - 1),
                            perf_mode=DR,
                        )
                        i_mm[j] += 1

                    if term_major:
                        for t_idx in range(3):
                            for j in range(4):
                                for P in range(NPAIR):
                                    for half in range(2):
                                        one(j, t_idx, P, half)
                    else:
                        for j in range(4):
                            for t_idx in range(3):
                                for P in range(NPAIR):
                                    for half in range(2):
                                        one(j, t_idx, P, half)

                def emit_qkv_group(g):
                    gs = slice(4 * g, 4 * (g + 1))
                    xh_b = xpool.tile([128, 4, NCH, 128], fp8,
                                      tag=f"xh{g % 2}", name=f"xh_b{g}")
                    xl_b = xpool.tile([128, 4, NCH, 128], fp8,
                                      tag=f"xl{g % 2}", name=f"xl_b{g}")
                    if g == 0:
                        # critical order: first matmul wave needs xh+wh only
                        nc.sync.dma_start(xh_b[:], xh_d[g])
                        nc.sync.dma_start(wh_sb[:], wh_d[:])
                        nc.sync.dma_start(xl_b[:], xl_d[g])
                        nc.sync.dma_start(wl_sb[:], wl_d[:])
                        nc.sync.dma_start(ve_sb[:], ve_d[:])
                        nc.sync.dma_start(cst_sb[:], cst_d[:])
                    else:
                        nc.sync.dma_start(xh_b[:], xh_d[g])
                        nc.sync.dma_start(xl_b[:], xl_d[g])
                    ps_tiles = [pq.tile([128, 3 * HD], f32, tag="qkvp",
                                        name=f"ps{g}_{j}") for j in range(4)]
                    emit_qkv_mms(g, ps_tiles, xh_b, xl_b, term_major=(g == 0))
                    dma_order_for_group(g)
                    sq_g = wpool.tile([128, 4, 2 * HD], f16, tag="sqg",
                                      name=f"sq_{g}")
                    for j in range(4):
                        ti = 4 * g + j
                        ps = ps_tiles[j]
                        # qk drain f32->f16 with 1/WS descale: ACT/DVE split
                        if ti % 2 == 0:
                            nc.scalar.activation(qkv[:, ti, 0:2 * HD],
                                                 ps[:, 0:2 * HD], AF.Copy,
                                                 bias=0.0, scale=1.0 / WS)
                        else:
                            nc.vector.tensor_scalar_mul(qkv[:, ti, 0:2 * HD],
                                                        ps[:, 0:2 * HD],
                                                        1.0 / WS)
                        # squares of q,k (f16, DVE 2x mode)
                        nc.vector.tensor_mul(sq_g[:, j, :],
                                             qkv[:, ti, 0:2 * HD],
                                             qkv[:, ti, 0:2 * HD])
                    # v drain fused with ve add (ve prescaled by lambda1;
                    # lambda0 folded into W plane scale): v = ps_v/WS + ve.
                    # Emitted before squares/reduce so the qkv PSUM banks
                    # release promptly (phase-2 score tiles reuse them).
                    for j in range(4):
                        ti = 4 * g + j
                        nc.vector.scalar_tensor_tensor(
                            qkv[:, ti, 2 * HD:], ps_tiles[j][:, 2 * HD:],
                            1.0 / WS, ve_sb[:, ti, :],
                            op0=ALU.mult, op1=ALU.add)
                    # per-group sumsq reduce: [128,4,256]->[128,8,128]->X
                    nc.vector.tensor_reduce(
                        rs[:, gs, :].rearrange("p a b -> p (a b)"),
                        sq_g[:].rearrange("p a (two d) -> p (a two) d", two=2),
                        axis=mybir.AxisListType.X,
                        op=ALU.add,
                    )
                    # rq = SCALE/sqrt(ms+eps), rk = 1/sqrt(ms+eps), computed
                    # as rsqrt on DVE (bit-trick seed + 2 Newton steps) so
                    # ACT never needs the sqrt table (one act-table load for
                    # the whole program).
                    xq = rx[:, gs, 0]
                    xk = rx[:, gs, 1]
                    nc.vector.tensor_scalar(xq, rs[:, gs, 0],
                                            1.0 / (HD * SCALE * SCALE),
                                            EPS / (SCALE * SCALE),
                                            op0=ALU.mult, op1=ALU.add)
                    nc.vector.tensor_scalar(xk, rs[:, gs, 1], 1.0 / HD,
                                            EPS, op0=ALU.mult, op1=ALU.add)
                    xz = rx[:, gs, :]
                    z = rs[:, gs, :]
                    t2 = rt[:, gs, :]
                    i32 = mybir.dt.int32
                    # seed: y0 = bits(0x5f3759df - (bits(x) >> 1))
                    nc.vector.tensor_scalar(z.bitcast(i32), xz.bitcast(i32),
                                            1, None,
                                            op0=ALU.arith_shift_right)
                    nc.vector.tensor_scalar(z.bitcast(i32), z.bitcast(i32),
                                            0xFFFFFFFF, 0x5f3759df + 1,
                                            op0=ALU.bitwise_xor, op1=ALU.add)
                    for _ in range(2):   # newton: y *= 1.5 - 0.5*x*y*y
                        nc.vector.tensor_mul(t2, z, z)
                        nc.vector.tensor_mul(t2, t2, xz)
                        nc.vector.tensor_scalar(t2, t2, -0.5, 1.5,
                                                op0=ALU.mult, op1=ALU.add)
                        nc.vector.tensor_mul(z, z, t2)

                def emit_tr_group(g):
                    hs = slice(4 * g, 4 * (g + 1))
                    for base in (0, HD):
                        eng = nc.vector if base == 0 else nc.gpsimd
                        x1 = qkv[:, hs, base + 0:base + 32]
                        x2 = qkv[:, hs, base + 64:base + 96]
                        t1 = wpool.tile([128, 4, 32], f16, tag=f"rot1{base}",
                                        name=f"t1_{base}")
                        t2 = wpool.tile([128, 4, 32], f16, tag=f"rot2{base}",
                                        name=f"t2_{base}")
                        t3 = wpool.tile([128, 4, 32], f16, tag=f"rot3{base}",
                                        name=f"t3_{base}")
                        t4 = wpool.tile([128, 4, 32], f16, tag=f"rot4{base}",
                                        name=f"t4_{base}")
                        eng.tensor_mul(t1[:], x1, cos_g(g))
                        eng.tensor_mul(t2[:], x2, sin_g(g))
                        eng.tensor_mul(t3[:], x2, cos_g(g))
                        eng.tensor_mul(t4[:], x1, sin_g(g))
                        eng.tensor_add(x1, t1[:], t2[:])
                        eng.tensor_sub(x2, t3[:], t4[:])
                    # normalize in place: q *= rq[t], k *= rk[t] (per-token
                    # scalar AP exempt from the DVE 2x-mode dtype check)
                    for j in range(4):
                        ti = 4 * g + j
                        nc.vector.tensor_scalar_mul(qkv[:, ti, 0:HD],
                                                    qkv[:, ti, 0:HD],
                                                    rs[:, ti, 0:1])
                        nc.vector.tensor_scalar_mul(qkv[:, ti, HD:2 * HD],
                                                    qkv[:, ti, HD:2 * HD],
                                                    rs[:, ti, 1:2])
                    # plain transposes (PE), drains on Pool
                    for base, isq in ((0, True), (HD, False)):
                        tp = ptr.tile([128, 4, 128], f32, tag="trp")
                        for j in range(4):
                            ti = 4 * g + j
                            nc.tensor.matmul(
                                tp[:, j, :],
                                qkv[:, ti, base:base + HD],
                                idn_v, start=True, stop=True)
                        dview = qT_sb[:, 512 * g:512 * (g + 1)] if isq \
                            else kT_sb[:, 4 * g:4 * (g + 1), :] \
                            .rearrange("p a b -> p (a b)")
                        tpf = tp[:].rearrange("p a b -> p (a b)")
                        nc.gpsimd.tensor_copy(dview, tpf)

                # interleave: qkv(g) then transposes(g-1) keeps PE busy
                for g in range(4):
                    emit_qkv_group(g)
                    if g >= 1:
                        emit_tr_group(g - 1)
                emit_tr_group(3)

            # ============ phase 2: attention + c_proj =====================
            with tc.tile_pool(name="ps_sc", bufs=2, space="PSUM") as psc, \
                 tc.tile_pool(name="ps_dn", bufs=1, space="PSUM") as pdn, \
                 tc.tile_pool(name="ps_y", bufs=1, space="PSUM") as py, \
                 tc.tile_pool(name="ps_o", bufs=2, space="PSUM") as po, \
                 tc.tile_pool(name="fin", bufs=2) as fin, \
                 tc.tile_pool(name="stp", bufs=1) as stp:

                def act_copy(dst, src):
                    nc.scalar.copy(dst, src)

                def dve_copy(dst, src):
                    nc.vector.tensor_copy(dst, src)

                def pool_copy(dst, src):
                    nc.gpsimd.tensor_copy(dst, src)

                drain_steady = [act_copy, dve_copy, pool_copy, pool_copy,
                                act_copy, dve_copy, pool_copy, pool_copy]
                # tail block avoids slow Pool copies entirely
                drain_tail = [act_copy, dve_copy, act_copy, dve_copy,
                              act_copy, dve_copy, act_copy, dve_copy]

                def gen_scores(tj):
                    pT_sb = pT_bufs[tj % 2]
                    t0 = TJ * tj
                    n_act = 4 * (tj + 1)
                    for p in range(n_act // 2):
                        sc2 = psc.tile([128, 2, TJ], f32, tag="scp",
                                       name=f"sc2_{tj}_{p}")
                        offs = []
                        for b in range(2):
                            si = 2 * p + b
                            o = si - 4 * tj
                            off = 128 * o if o > 0 else 0
                            offs.append(off)
                            nc.tensor.matmul(
                                sc2[:, b, 0:TJ - off], kT_sb[:, si, :],
                                qT_sb[:, t0 + off:t0 + TJ],
                                start=True, stop=True)
                            yield
                        # paired exp over both banks (garbage regions of
                        # diagonal blocks are exp'd too but never read)
                        nc.scalar.activation(
                            pT_sb[:, 2 * p:2 * p + 2, :], sc2[:], AF.Exp,
                            bias=bexp_sb[:])
                    # merged diagonal triangle mask (DVE)
                    import concourse.bass as bass
                    base = pT_sb[:, 4 * tj, 0:128]
                    mview = bass.AP(
                        tensor=base.tensor, offset=base.offset,
                        ap=[list(base.ap[0]), [TJ + 128, 4], [1, 128]])
                    tri_b = _bcast_mid(tri_v, 4)
                    nc.vector.tensor_tensor(mview, mview, tri_b, op=ALU.mult)

                def gen_av(tj, out_yh):
                    pT_sb = pT_bufs[tj % 2]
                    n_act = 4 * (tj + 1)
                    dn = pdn.tile([128, TJ], f32, tag="dn")
                    yT = py.tile([128, TJ], f32, tag="yT")
                    for si in range(n_act):
                        o = si - 4 * tj
                        off = 128 * o if o > 0 else 0
                        first = (si == 0)
                        last = (si == n_act - 1)
                        nc.tensor.matmul(dn[:, off:TJ], ones_sb[:],
                                         pT_sb[:, si, off:TJ],
                                         start=first, stop=last)
                        yield
                        nc.tensor.matmul(yT[:, off:TJ],
                                         qkv[:, si, 2 * HD:],
                                         pT_sb[:, si, off:TJ],
                                         start=first, stop=last)
                        yield
                    rdn = fin.tile([128, TJ], f32, tag=f"rdn{tj % 2}",
                                   name=f"rdn_{tj}")
                    nc.vector.reciprocal(rdn[:], dn[:])
                    yh = fin.tile([128, TJ], f16, tag=f"yh{tj % 2}",
                                  name=f"yh_{tj}")
                    nc.vector.tensor_mul(yh[:], yT[:], rdn[:])
                    out_yh.append(yh)

                def gen_cproj(tj, yh):
                    t0 = TJ * tj
                    st = stp.tile([128, 8, TJ], f16, tag=f"st{tj % 2}",
                                  name=f"st_{tj}")
                    import concourse.bass as bass
                    e_per = 2 if tj == 3 else 4
                    drains = drain_tail if tj == 3 else drain_steady
                    for e in range(8):
                        pot = po.tile([128, TJ], f32, tag="pot")
                        nc.tensor.matmul(pot[:],
                                         cw_sb[:, 128 * e:128 * (e + 1)],
                                         yh[:],
                                         start=True, stop=True)
                        yield
                        drains[e](st[:, e, :], pot[:])
                        if e % e_per == e_per - 1:
                            e0 = e - e_per + 1
                            dst = out_d[0:128, t0:t0 + TJ]
                            dstb = bass.AP(
                                tensor=dst.tensor,
                                offset=dst.offset + 128 * e0 * T,
                                ap=[list(dst.ap[0]), [128 * T, e_per],
                                    list(dst.ap[1])])
                            nc.sync.dma_start(dstb, st[:, e0:e0 + e_per, :])

                # pipeline: scores two blocks ahead; cproj matmuls ride
                # between score/av matmuls so the PE queue never stalls on
                # the 2-buf cproj PSUM drains.
                yhs = []
                _drive(gen_scores(0))
                _drive(gen_scores(1))
                _drive(gen_av(0, yhs))
                _drive(gen_scores(2), gen_cproj(0, yhs[0]))
                _drive(gen_av(1, yhs))
                _drive(gen_scores(3), gen_cproj(1, yhs[1]))
                _drive(gen_av(2, yhs))
                _drive(gen_av(3, yhs), gen_cproj(2, yhs[2]))
                _drive(gen_cproj(3, yhs[3]))

    nc.compile()
    _CACHE["nc"] = nc
    return nc


def _host_inputs(x, ve, qkv_w, lambdas, c_proj_w):
    """Build the 8 per-core input maps (layout transforms only)."""
    import ml_dtypes
    f16 = ml_dtypes.float16 if hasattr(ml_dtypes, "float16") else np.float16
    e4 = ml_dtypes.float8_e4m3
    x = np.asarray(x, np.float32)
    ve = np.asarray(ve, np.float32)
    qkv_w = np.asarray(qkv_w, np.float32)
    lambdas = np.asarray(lambdas, np.float32)
    c_proj_w = np.asarray(c_proj_w, np.float32)

    # x planes: [4 batch, 128 cin, 4 tile, NCH, 128 tok]
    xr = x[0].reshape(NT, 128, NCH, 128).transpose(0, 3, 2, 1)
    xr = xr.reshape(4, 4, 128, NCH, 128).transpose(0, 2, 1, 3, 4)
    xh = xr.astype(e4)
    xl = (xr - xh.astype(np.float32)).astype(e4)
    xh = np.ascontiguousarray(xh)
    xl = np.ascontiguousarray(xl)

    freq = (1.0 / 1024.0) ** np.linspace(0.0, 1.0, HD // 4, dtype=np.float32)
    theta = np.arange(T, dtype=np.float32)[:, None] * freq[None, :]  # [T, 32]
    cosT = np.cos(theta).astype(f16).reshape(NT, 128, 32).transpose(1, 0, 2)
    sinT = np.sin(theta).astype(f16).reshape(NT, 128, 32).transpose(1, 0, 2)
    tri = (np.arange(128)[None, :] >= np.arange(128)[:, None]).astype(f16)
    idn = np.eye(128, dtype=np.float32).astype(f16)
    cst = np.ascontiguousarray(np.concatenate([
        cosT.reshape(128, 512), sinT.reshape(128, 512), tri, idn,
    ], axis=1))  # [128, 1280]

    lam0, lam1 = float(lambdas[0]), float(lambdas[1])
    wscale = np.concatenate([np.full(2 * HD, WS, np.float32),
                             np.full(HD, WS * lam0, np.float32)])

    in_maps = []
    for h in range(NCORES):
        sl = slice(128 * h, 128 * (h + 1))
        # W planes: [128 cin, NCH, 3*HD], prescaled
        whd = qkv_w[:, sl, :]                          # [3, 128, 1024]
        wt = whd.transpose(2, 0, 1).reshape(D, 3 * HD)  # [cin-full, 384]
        wt = wt * wscale[None, :]
        wt = wt.reshape(NCH, 128, 3 * HD).transpose(1, 0, 2)  # [128, NCH, 384]
        wh = wt.astype(e4)
        wl = (wt - wh.astype(np.float32)).astype(e4)
        wh = np.ascontiguousarray(wh)
        wl = np.ascontiguousarray(wl)
        veh = np.ascontiguousarray(
            (lam1 * ve[0, :, sl]).reshape(NT, 128, HD)
            .transpose(1, 0, 2).astype(f16))
        cwh = np.ascontiguousarray(c_proj_w[:, sl].T.astype(f16))  # [128, 1024]
        in_maps.append({
            "xh": xh, "xl": xl, "wh": wh, "wl": wl, "veN": veh, "cwT": cwh,
            "cst": cst,
        })
    return in_maps


def run(x, ve, qkv_w, lambdas, c_proj_w, trace=False):
    from concourse.bass_utils import run_bass_kernel_spmd

    nc = _build_program()
    in_maps = _host_inputs(x, ve, qkv_w, lambdas, c_proj_w)
    res = run_bass_kernel_spmd(
        nc, in_maps, core_ids=list(range(NCORES)), trace=trace)
    acc = np.zeros((D, T), np.float64)
    for r in res.results:
        acc += r["outT"].astype(np.float64)
    out = acc.astype(np.float32).T.reshape(B, T, D)
    return out, res


def kernel(x, ve, qkv_w, lambdas, c_proj_w):
    out, _ = run(x, ve, qkv_w, lambdas, c_proj_w, trace=False)
    return out


# revision 34
# speedup vs baseline: 1.1016x; 1.0142x over previous
"""Causal self-attention (B=1, T=2048, D=1024, H=8, hd=128) on 8 trn2 cores.

Sharding: tensor-parallel over heads -- one head per core. Each core computes
its head's qkv projection, rms-norm+rotary, causal attention, and the c_proj
partial product for its head; the host sums the 8 partial [D, T] outputs.

v2 restructure (from the 81.1us baseline), engine-balance driven:
  - ACT diet: ACT keeps only Exp (paired 2-bank PSUM score tiles -> 20 exps
    instead of 40), Sqrt (8 tiny), half the qkv drains, and a share of the
    c_proj drains. Everything else moved to DVE/Pool. A scheduling-only dep
    forces all Sqrts before the first Exp so exactly 2 act-table loads are
    emitted (was 5 x 1283ns with mid-phase thrash).
  - squares+reduce fused into DVE tensor_tensor_reduce (accum_out) per tile;
    v-drain fused with the ve add via scalar_tensor_tensor.
  - diag(rq)/diag(rk) builds on DVE (f16 2x mode) instead of Pool.
  - DMA: first qkv group is term-major (all (xh,wh) matmuls for 4 tiles, then
    (xl,wh), then (xh,wl)) so PE starts right after xh+wh land (~4.5us);
    consts packed into one blob DMA; load order xh0,wh,xl0,wl,ve,cst,...
  - c_proj drains round-robin ACT/DVE/Pool; c_proj PE matmuls interleaved
    with score/av matmuls so the in-order PE queue never stalls on the
    2-buf PSUM drain; output stored in half-blocks (quarters for the last
    t-block) to shrink the tail.
"""

import numpy as np

B, T, D = 1, 2048, 1024
H, HD = 8, 128
SCALE = 0.12
NCORES = 8
NT = T // 128      # 16 token tiles
NCH = D // 128     # 8 contraction chunks
NPAIR = NCH // 2   # 4 DoubleRow chunk pairs
NTJ = 4            # attention t-blocks
TJ = T // NTJ      # 512
EPS = float(np.finfo(np.float32).eps)
WS = 64.0          # fp8 weight prescale
ESH = 4.5          # exp shift

_CACHE = {}


def _bcast(ap, n):
    """Broadcast a [..., 1] AP to [..., n] via a step-0 trailing dim."""
    try:
        return ap.to_broadcast(list(ap.shape[:-1]) + [n])
    except Exception:
        import concourse.bass as bass
        return bass.AP(tensor=ap.tensor, offset=ap.offset,
                       ap=list(ap.ap[:-1]) + [[0, n]])


def _bcast_mid(ap, n):
    """Insert a step-0 middle dim: [p, f] -> [p, n, f]."""
    import concourse.bass as bass
    return bass.AP(tensor=ap.tensor, offset=ap.offset,
                   ap=[list(ap.ap[0]), [0, n], list(ap.ap[1])])


def _view(tile_ap, start, dims):
    """View into a flat [128, N] AP at elem offset `start` with free dims
    [(stride, count), ...]."""
    import concourse.bass as bass
    return bass.AP(tensor=tile_ap.tensor, offset=tile_ap.offset + start,
                   ap=[list(tile_ap.ap[0])] + [list(d) for d in dims])


def _drive(*gens):
    """Round-robin drive generators to completion (interleaves PE work)."""
    gens = [g for g in gens if g is not None]
    while gens:
        nxt = []
        for g in gens:
            try:
                next(g)
                nxt.append(g)
            except StopIteration:
                pass
        gens = nxt


def _build_program():
    if "nc" in _CACHE:
        return _CACHE["nc"]

    import concourse.bacc as bacc
    import concourse.tile as tile
    import concourse.mybir as mybir
    from concourse.tile_rust import add_dep_helper

    f32 = mybir.dt.float32
    f16 = mybir.dt.float16
    fp8 = mybir.dt.float8e4
    AF = mybir.ActivationFunctionType
    ALU = mybir.AluOpType
    DR = mybir.MatmulPerfMode.DoubleRow

    nc = bacc.Bacc("TRN2", target_bir_lowering=False, debug=False)

    # x planes: [batch of 4 tiles, c-in-part 128, tile, chunk, token]
    xh_d = nc.dram_tensor("xh", [4, 128, 4, NCH, 128], fp8,
                          kind="ExternalInput")
    xl_d = nc.dram_tensor("xl", [4, 128, 4, NCH, 128], fp8,
                          kind="ExternalInput")
    wh_d = nc.dram_tensor("wh", [128, NCH, 3 * HD], fp8, kind="ExternalInput")
    wl_d = nc.dram_tensor("wl", [128, NCH, 3 * HD], fp8, kind="ExternalInput")
    ve_d = nc.dram_tensor("veN", [128, NT, HD], f16, kind="ExternalInput")
    cw_d = nc.dram_tensor("cwT", [HD, D], f16, kind="ExternalInput")
    # packed consts per partition: cos(512) | sin(512) | tri(128) | idn(128)
    cst_d = nc.dram_tensor("cst", [128, 1280], f16, kind="ExternalInput")
    out_d = nc.dram_tensor("outT", [D, T], f16, kind="ExternalOutput")

    with tile.TileContext(nc) as tc:
        with tc.tile_pool(name="const", bufs=1) as cpool, \
             tc.tile_pool(name="work", bufs=1) as wpool, \
             tc.tile_pool(name="xs", bufs=1) as xpool:
            # ---- resident inputs ----
            wh_sb = cpool.tile([128, NCH, 3 * HD], fp8)
            wl_sb = cpool.tile([128, NCH, 3 * HD], fp8)
            ve_sb = cpool.tile([128, NT, HD], f16)
            cw_sb = cpool.tile([HD, D], f16)
            cst_sb = cpool.tile([128, 1280], f16)
            ones_sb = cpool.tile([128, 128], f16)
            bexp_sb = cpool.tile([128, 1], f32)   # exp shift bias

            cstf = cst_sb[:]

            def cos_g(g):     # [128, 4, 32] for tile group g
                return _view(cstf, 128 * g, [[32, 4], [1, 32]])

            def sin_g(g):
                return _view(cstf, 512 + 128 * g, [[32, 4], [1, 32]])

            tri_v = _view(cstf, 1024, [[1, 128]])
            idn_v = _view(cstf, 1152, [[1, 128]])

            nc.vector.memset(ones_sb[:], 1.0)
            nc.vector.memset(bexp_sb[:], -ESH)

            # ---- working buffers ----
            qkv = wpool.tile([128, NT, 3 * HD], f16)      # natural qkv
            rs = wpool.tile([128, NT, 2], f32)            # rms scalars q,k
            rx = wpool.tile([128, NT, 2], f32)            # rsqrt arg scratch
            rt = wpool.tile([128, NT, 2], f32)            # newton scratch
            qT_sb = wpool.tile([128, T], f16)             # q-hat.T [d, t]
            kT_sb = wpool.tile([128, NT, 128], f16)       # k-hat.T [d, si, s']
            pT_bufs = [wpool.tile([128, NT, TJ], f16, tag=f"pT{i}",
                                  name=f"pT{i}") for i in range(2)]

            # ============ phase 1: qkv (fp8 DoubleRow) + rms + rotary ======
            with tc.tile_pool(name="ps_qkv", bufs=6, space="PSUM") as pq, \
                 tc.tile_pool(name="ps_tr", bufs=2, space="PSUM") as ptr:

                def dma_order_for_group(g):
                    # interleave remaining loads so x batches stay critical
                    if g == 2:
                        nc.sync.dma_start(cw_sb[:], cw_d[:])

                def emit_qkv_mms(g, ps_tiles, xh_b, xl_b, term_major):
                    """24 DR matmuls per tile; term-major runs the 3 planes
                    as waves across all 4 tiles (x/w DMA arrival order)."""
                    terms = ((xh_b, wh_sb), (xl_b, wh_sb), (xh_b, wl_sb))
                    i_mm = [0] * 4
                    n_mm = NPAIR * 3 * 2

                    def one(j, t_idx, P, half):
                        xa, wa = terms[t_idx]
                        cs = slice(2 * P, 2 * P + 2)
                        hs2 = slice(192 * half, 192 * (half + 1))
                        nc.tensor.matmul(
                            ps_tiles[j][:, hs2],
                            xa[:, j, cs, :],
                            wa[:, cs, hs2],
                            start=(i_mm[j] == 0),
                            stop=(i_mm[j] == n_mm - 1),
                            perf_mode=DR,
                        )
                        i_mm[j] += 1

                    if term_major:
                        for t_idx in range(3):
                            for j in range(4):
                                for P in range(NPAIR):
                                    for half in range(2):
                                        one(j, t_idx, P, half)
                    else:
                        for j in range(4):
                            for t_idx in range(3):
                                for P in range(NPAIR):
                                    for half in range(2):
                                        one(j, t_idx, P, half)

                def emit_qkv_group(g):
                    gs = slice(4 * g, 4 * (g + 1))
                    xh_b = xpool.tile([128, 4, NCH, 128], fp8,
                                      tag=f"xh{g % 2}", name=f"xh_b{g}")
                    xl_b = xpool.tile([128, 4, NCH, 128], fp8,
                                      tag=f"xl{g % 2}", name=f"xl_b{g}")
                    if g == 0:
                        # critical order: first matmul wave needs xh+wh only
                        nc.sync.dma_start(xh_b[:], xh_d[g])
                        nc.sync.dma_start(wh_sb[:], wh_d[:])
                        nc.sync.dma_start(xl_b[:], xl_d[g])
                        nc.sync.dma_start(wl_sb[:], wl_d[:])
                        nc.sync.dma_start(ve_sb[:], ve_d[:])
                        nc.sync.dma_start(cst_sb[:], cst_d[:])
                    else:
                        nc.sync.dma_start(xh_b[:], xh_d[g])
                        nc.sync.dma_start(xl_b[:], xl_d[g])
                    ps_tiles = [pq.tile([128, 3 * HD], f32, tag="qkvp",
                                        name=f"ps{g}_{j}") for j in range(4)]
                    emit_qkv_mms(g, ps_tiles, xh_b, xl_b, term_major=(g == 0))
                    dma_order_for_group(g)
                    sq_g = wpool.tile([128, 4, 2 * HD], f16, tag="sqg",
                                      name=f"sq_{g}")
                    for j in range(4):
                        ti = 4 * g + j
                        ps = ps_tiles[j]
                        # qk drain f32->f16 with 1/WS descale: ACT/DVE split
                        if ti % 2 == 0:
                            nc.scalar.activation(qkv[:, ti, 0:2 * HD],
                                                 ps[:, 0:2 * HD], AF.Copy,
                                                 bias=0.0, scale=1.0 / WS)
                        else:
                            nc.vector.tensor_scalar_mul(qkv[:, ti, 0:2 * HD],
                                                        ps[:, 0:2 * HD],
                                                        1.0 / WS)
                        # squares of q,k (f16, SBUF-only -> Pool)
                        nc.gpsimd.tensor_mul(sq_g[:, j, :],
                                             qkv[:, ti, 0:2 * HD],
                                             qkv[:, ti, 0:2 * HD])
                    # v drain fused with ve add (ve prescaled by lambda1;
                    # lambda0 folded into W plane scale): v = ps_v/WS + ve.
                    # Emitted before squares/reduce so the qkv PSUM banks
                    # release promptly (phase-2 score tiles reuse them).
                    for j in range(4):
                        ti = 4 * g + j
                        nc.vector.scalar_tensor_tensor(
                            qkv[:, ti, 2 * HD:], ps_tiles[j][:, 2 * HD:],
                            1.0 / WS, ve_sb[:, ti, :],
                            op0=ALU.mult, op1=ALU.add)
                    # per-group sumsq reduce: [128,4,256]->[128,8,128]->X
                    nc.vector.tensor_reduce(
                        rs[:, gs, :].rearrange("p a b -> p (a b)"),
                        sq_g[:].rearrange("p a (two d) -> p (a two) d", two=2),
                        axis=mybir.AxisListType.X,
                        op=ALU.add,
                    )
                    # rq = SCALE/sqrt(ms+eps), rk = 1/sqrt(ms+eps), computed
                    # as rsqrt on DVE (bit-trick seed + 2 Newton steps) so
                    # ACT never needs the sqrt table (one act-table load for
                    # the whole program).
                    xq = rx[:, gs, 0]
                    xk = rx[:, gs, 1]
                    nc.vector.tensor_scalar(xq, rs[:, gs, 0],
                                            1.0 / (HD * SCALE * SCALE),
                                            EPS / (SCALE * SCALE),
                                            op0=ALU.mult, op1=ALU.add)
                    nc.vector.tensor_scalar(xk, rs[:, gs, 1], 1.0 / HD,
                                            EPS, op0=ALU.mult, op1=ALU.add)
                    xz = rx[:, gs, :]
                    z = rs[:, gs, :]
                    t2 = rt[:, gs, :]
                    i32 = mybir.dt.int32
                    # seed: y0 = bits(0x5f3759df - (bits(x) >> 1))
                    nc.vector.tensor_scalar(z.bitcast(i32), xz.bitcast(i32),
                                            1, None,
                                            op0=ALU.arith_shift_right)
                    nc.vector.tensor_scalar(z.bitcast(i32), z.bitcast(i32),
                                            -1, None,
                                            op0=ALU.bitwise_xor)
                    nc.vector.tensor_scalar(z.bitcast(i32), z.bitcast(i32),
                                            0x5f3759df + 1, None,
                                            op0=ALU.add)
                    for _ in range(2):   # newton: y *= 1.5 - 0.5*x*y*y
                        nc.vector.tensor_mul(t2, z, z)
                        nc.vector.tensor_mul(t2, t2, xz)
                        nc.vector.tensor_scalar(t2, t2, -0.5, 1.5,
                                                op0=ALU.mult, op1=ALU.add)
                        nc.vector.tensor_mul(z, z, t2)

                def emit_tr_group(g):
                    hs = slice(4 * g, 4 * (g + 1))
                    for base in (0, HD):
                        eng = nc.vector if base == 0 else nc.gpsimd
                        x1 = qkv[:, hs, base + 0:base + 32]
                        x2 = qkv[:, hs, base + 64:base + 96]
                        t1 = wpool.tile([128, 4, 32], f16, tag=f"rot1{base}",
                                        name=f"t1_{base}")
                        t2 = wpool.tile([128, 4, 32], f16, tag=f"rot2{base}",
                                        name=f"t2_{base}")
                        t3 = wpool.tile([128, 4, 32], f16, tag=f"rot3{base}",
                                        name=f"t3_{base}")
                        t4 = wpool.tile([128, 4, 32], f16, tag=f"rot4{base}",
                                        name=f"t4_{base}")
                        eng.tensor_mul(t1[:], x1, cos_g(g))
                        eng.tensor_mul(t2[:], x2, sin_g(g))
                        eng.tensor_mul(t3[:], x2, cos_g(g))
                        eng.tensor_mul(t4[:], x1, sin_g(g))
                        eng.tensor_add(x1, t1[:], t2[:])
                        eng.tensor_sub(x2, t3[:], t4[:])
                    # normalize in place: q *= rq[t], k *= rk[t] (per-token
                    # scalar AP exempt from the DVE 2x-mode dtype check)
                    for j in range(4):
                        ti = 4 * g + j
                        nc.vector.tensor_scalar_mul(qkv[:, ti, 0:HD],
                                                    qkv[:, ti, 0:HD],
                                                    rs[:, ti, 0:1])
                        nc.vector.tensor_scalar_mul(qkv[:, ti, HD:2 * HD],
                                                    qkv[:, ti, HD:2 * HD],
                                                    rs[:, ti, 1:2])
                    # plain transposes (PE), drains on Pool
                    for base, isq in ((0, True), (HD, False)):
                        tp = ptr.tile([128, 4, 128], f32, tag="trp")
                        for j in range(4):
                            ti = 4 * g + j
                            nc.tensor.matmul(
                                tp[:, j, :],
                                qkv[:, ti, base:base + HD],
                                idn_v, start=True, stop=True)
                        dview = qT_sb[:, 512 * g:512 * (g + 1)] if isq \
                            else kT_sb[:, 4 * g:4 * (g + 1), :] \
                            .rearrange("p a b -> p (a b)")
                        tpf = tp[:].rearrange("p a b -> p (a b)")
                        # PSUM drains are ACT/DVE only (gpsimd can't)
                        if isq == (g % 2 == 0):
                            nc.scalar.copy(dview, tpf)
                        else:
                            nc.vector.tensor_copy(dview, tpf)

                # interleave: qkv(g) then transposes(g-1) keeps PE busy
                for g in range(4):
                    emit_qkv_group(g)
                    if g >= 1:
                        emit_tr_group(g - 1)
                emit_tr_group(3)

            # ============ phase 2: attention + c_proj =====================
            with tc.tile_pool(name="ps_sc", bufs=2, space="PSUM") as psc, \
                 tc.tile_pool(name="ps_dn", bufs=1, space="PSUM") as pdn, \
                 tc.tile_pool(name="ps_y", bufs=1, space="PSUM") as py, \
                 tc.tile_pool(name="ps_o", bufs=2, space="PSUM") as po, \
                 tc.tile_pool(name="fin", bufs=2) as fin, \
                 tc.tile_pool(name="stp", bufs=1) as stp:

                def act_copy(dst, src):
                    nc.scalar.copy(dst, src)

                def dve_copy(dst, src):
                    nc.vector.tensor_copy(dst, src)

                def pool_copy(dst, src):
                    nc.gpsimd.tensor_copy(dst, src)

                # PSUM drains are ACT/DVE only (gpsimd can't touch PSUM)
                drain_steady = [act_copy, dve_copy, act_copy, dve_copy,
                                act_copy, dve_copy, act_copy, dve_copy]
                drain_tail = drain_steady

                def gen_scores(tj):
                    pT_sb = pT_bufs[tj % 2]
                    t0 = TJ * tj
                    n_act = 4 * (tj + 1)
                    for p in range(n_act // 2):
                        sc2 = psc.tile([128, 2, TJ], f32, tag="scp",
                                       name=f"sc2_{tj}_{p}")
                        # diagonal blocks compute their full row too (the
                        # sub-diagonal part is finite garbage, never read by
                        # dn/yT) so the paired exp reads fully-written PSUM
                        for b in range(2):
                            si = 2 * p + b
                            nc.tensor.matmul(
                                sc2[:, b, :], kT_sb[:, si, :],
                                qT_sb[:, t0:t0 + TJ],
                                start=True, stop=True)
                            yield
                        # paired exp over both banks (garbage regions of
                        # diagonal blocks are exp'd too but never read)
                        nc.scalar.activation(
                            pT_sb[:, 2 * p:2 * p + 2, :], sc2[:], AF.Exp,
                            bias=bexp_sb[:])
                    # merged diagonal triangle mask (DVE)
                    import concourse.bass as bass
                    base = pT_sb[:, 4 * tj, 0:128]
                    mview = bass.AP(
                        tensor=base.tensor, offset=base.offset,
                        ap=[list(base.ap[0]), [TJ + 128, 4], [1, 128]])
                    tri_b = _bcast_mid(tri_v, 4)
                    # SBUF-only -> Pool (keeps ACT/DVE free for drains)
                    nc.gpsimd.tensor_tensor(mview, mview, tri_b, op=ALU.mult)

                def gen_av(tj, out_yh):
                    pT_sb = pT_bufs[tj % 2]
                    n_act = 4 * (tj + 1)
                    dn = pdn.tile([128, TJ], f32, tag="dn")
                    yT = py.tile([128, TJ], f32, tag="yT")
                    for si in range(n_act):
                        o = si - 4 * tj
                        off = 128 * o if o > 0 else 0
                        first = (si == 0)
                        last = (si == n_act - 1)
                        nc.tensor.matmul(dn[:, off:TJ], ones_sb[:],
                                         pT_sb[:, si, off:TJ],
                                         start=first, stop=last)
                        yield
                        nc.tensor.matmul(yT[:, off:TJ],
                                         qkv[:, si, 2 * HD:],
                                         pT_sb[:, si, off:TJ],
                                         start=first, stop=last)
                        yield
                    rdn = fin.tile([128, TJ], f32, tag=f"rdn{tj % 2}",
                                   name=f"rdn_{tj}")
                    nc.vector.reciprocal(rdn[:], dn[:])
                    yh = fin.tile([128, TJ], f16, tag=f"yh{tj % 2}",
                                  name=f"yh_{tj}")
                    nc.vector.tensor_mul(yh[:], yT[:], rdn[:])
                    out_yh.append(yh)

                def gen_cproj(tj, yh):
                    t0 = TJ * tj
                    st = stp.tile([128, 8, TJ], f16, tag=f"st{tj % 2}",
                                  name=f"st_{tj}")
                    import concourse.bass as bass
                    e_per = 2 if tj == 3 else 4
                    drains = drain_tail if tj == 3 else drain_steady
                    for e in range(8):
                        pot = po.tile([128, TJ], f32, tag="pot")
                        nc.tensor.matmul(pot[:],
                                         cw_sb[:, 128 * e:128 * (e + 1)],
                                         yh[:],
                                         start=True, stop=True)
                        yield
                        drains[e](st[:, e, :], pot[:])
                        if e % e_per == e_per - 1:
                            e0 = e - e_per + 1
                            dst = out_d[0:128, t0:t0 + TJ]
                            dstb = bass.AP(
                                tensor=dst.tensor,
                                offset=dst.offset + 128 * e0 * T,
                                ap=[list(dst.ap[0]), [128 * T, e_per],
                                    list(dst.ap[1])])
                            nc.sync.dma_start(dstb, st[:, e0:e0 + e_per, :])

                # pipeline: scores two blocks ahead; cproj matmuls ride
                # between score/av matmuls so the PE queue never stalls on
                # the 2-buf cproj PSUM drains.
                yhs = []
                _drive(gen_scores(0))
                _drive(gen_scores(1))
                _drive(gen_av(0, yhs))
                _drive(gen_scores(2), gen_cproj(0, yhs[0]))
                _drive(gen_av(1, yhs))
                _drive(gen_scores(3), gen_cproj(1, yhs[1]))
                _drive(gen_av(2, yhs))
                _drive(gen_av(3, yhs), gen_cproj(2, yhs[2]))
                _drive(gen_cproj(3, yhs[3]))

    nc.compile()
    _CACHE["nc"] = nc
    return nc


def _host_inputs(x, ve, qkv_w, lambdas, c_proj_w):
    """Build the 8 per-core input maps (layout transforms only)."""
    import ml_dtypes
    f16 = ml_dtypes.float16 if hasattr(ml_dtypes, "float16") else np.float16
    e4 = ml_dtypes.float8_e4m3
    x = np.asarray(x, np.float32)
    ve = np.asarray(ve, np.float32)
    qkv_w = np.asarray(qkv_w, np.float32)
    lambdas = np.asarray(lambdas, np.float32)
    c_proj_w = np.asarray(c_proj_w, np.float32)

    # x planes: [4 batch, 128 cin, 4 tile, NCH, 128 tok]
    xr = x[0].reshape(NT, 128, NCH, 128).transpose(0, 3, 2, 1)
    xr = xr.reshape(4, 4, 128, NCH, 128).transpose(0, 2, 1, 3, 4)
    xh = xr.astype(e4)
    xl = (xr - xh.astype(np.float32)).astype(e4)
    xh = np.ascontiguousarray(xh)
    xl = np.ascontiguousarray(xl)

    freq = (1.0 / 1024.0) ** np.linspace(0.0, 1.0, HD // 4, dtype=np.float32)
    theta = np.arange(T, dtype=np.float32)[:, None] * freq[None, :]  # [T, 32]
    cosT = np.cos(theta).astype(f16).reshape(NT, 128, 32).transpose(1, 0, 2)
    sinT = np.sin(theta).astype(f16).reshape(NT, 128, 32).transpose(1, 0, 2)
    tri = (np.arange(128)[None, :] >= np.arange(128)[:, None]).astype(f16)
    idn = np.eye(128, dtype=np.float32).astype(f16)
    cst = np.ascontiguousarray(np.concatenate([
        cosT.reshape(128, 512), sinT.reshape(128, 512), tri, idn,
    ], axis=1))  # [128, 1280]

    lam0, lam1 = float(lambdas[0]), float(lambdas[1])
    wscale = np.concatenate([np.full(2 * HD, WS, np.float32),
                             np.full(HD, WS * lam0, np.float32)])

    in_maps = []
    for h in range(NCORES):
        sl = slice(128 * h, 128 * (h + 1))
        # W planes: [128 cin, NCH, 3*HD], prescaled
        whd = qkv_w[:, sl, :]                          # [3, 128, 1024]
        wt = whd.transpose(2, 0, 1).reshape(D, 3 * HD)  # [cin-full, 384]
        wt = wt * wscale[None, :]
        wt = wt.reshape(NCH, 128, 3 * HD).transpose(1, 0, 2)  # [128, NCH, 384]
        wh = wt.astype(e4)
        wl = (wt - wh.astype(np.float32)).astype(e4)
        wh = np.ascontiguousarray(wh)
        wl = np.ascontiguousarray(wl)
        veh = np.ascontiguousarray(
            (lam1 * ve[0, :, sl]).reshape(NT, 128, HD)
            .transpose(1, 0, 2).astype(f16))
        cwh = np.ascontiguousarray(c_proj_w[:, sl].T.astype(f16))  # [128, 1024]
        in_maps.append({
            "xh": xh, "xl": xl, "wh": wh, "wl": wl, "veN": veh, "cwT": cwh,
            "cst": cst,
        })
    return in_maps


def run(x, ve, qkv_w, lambdas, c_proj_w, trace=False):
    from concourse.bass_utils import run_bass_kernel_spmd

    nc = _build_program()
    in_maps = _host_inputs(x, ve, qkv_w, lambdas, c_proj_w)
    res = run_bass_kernel_spmd(
        nc, in_maps, core_ids=list(range(NCORES)), trace=trace)
    acc = np.zeros((D, T), np.float64)
    for r in res.results:
        acc += r["outT"].astype(np.float64)
    out = acc.astype(np.float32).T.reshape(B, T, D)
    return out, res


def kernel(x, ve, qkv_w, lambdas, c_proj_w):
    out, _ = run(x, ve, qkv_w, lambdas, c_proj_w, trace=False)
    return out
